# revision 1
# baseline (speedup 1.0000x reference)
"""DCGRU cell on 8 Trainium2 NeuronCores.

Sharding: data-parallel over batch (B=32 -> 4 per core), adjacency + MLP
weights replicated. No collectives; host gathers per-core outputs.

Per-core layouts (all f32):
  node-major (nm): [16 tiles][128 nodes, 768] cols = b*192+f   (diffusion lhsT)
  feat-major (fm): [6 tiles][128 bf-rows, 2048 nodes]          (hop outputs, MLP rhs)
Hop matmul: out_fm[bf, i] = sum_j x_nm[j, bf] * W[i, j]
  = matmul(lhsT=x_nm[jt][:, c*128:+128], rhs=WT[jt][:, i-block]) accumulated
  over jt in PSUM, so W is streamed host-pretransposed (WT[j, i] = W[i, j]).
MLP: gate logits acc[b][o, n] += WxI[k][bf, o].T @ fm[k][bf-slice, n] with
  batch-interleaved host-packed weights WxI (rows = b*192+f), accumulated
  across hops in DRAM via accum_op=add DMAs straight from PSUM.
Chain re-entry: fm -> nm via PE transposes (hops 1,2 of each direction only).
"""

import sys
import numpy as np
import ml_dtypes

for _p in ("/opt/trn_rl_repo",):
    if _p not in sys.path:
        sys.path.insert(0, _p)

from concourse import bacc, tile, mybir  # noqa: E402
from concourse.alu_op_type import AluOpType as ALU  # noqa: E402
from concourse.bass_utils import run_bass_kernel_spmd  # noqa: E402

F32 = mybir.dt.float32
F32R = mybir.dt.float32r
BF16 = mybir.dt.bfloat16
MM_BF16 = True          # matmul datapath dtype: True -> bf16, False -> f32r
MMDT = BF16 if MM_BF16 else F32R
AF = mybir.ActivationFunctionType

C = 4          # batches per core
FI = 192       # per-batch feature width (x 64 + h 128)
BF = C * FI    # 768
DH = 128
NCORES = 8
NHOPS = 3


def build_nc(nt=16):
    """Build + compile the per-core Bass kernel. nt = node tiles (N = nt*128)."""
    N = nt * 128
    nbk = N // 512

    nc = bacc.Bacc("TRN2", target_bir_lowering=False, debug=False,
                   num_devices=NCORES)

    def din(name, shape, dt=F32):
        return nc.dram_tensor(name, shape, dt, kind="ExternalInput").ap()

    XH = din("xh_nm", [nt, 128, BF], MMDT)
    XHFM = din("xh_fm", [6, 128, N], MMDT)
    WFT = din("wfT", [nt, 128, N], MMDT)
    WBT = din("wbT", [nt, 128, N], MMDT)
    WRI = din("wrI", [7, 3, 64, 128], MMDT)
    WZI = din("wzI", [7, 3, 64, 128], MMDT)
    WNI = din("wnI", [7, 3, 64, 128], MMDT)
    XFM = din("x_fm", [C, 64, N], MMDT)
    HFM = din("h_fm", [C, 128, N])
    BR = din("br_c", [128, 1])
    BZ = din("bz_c", [128, 1])
    BN = din("bn_c", [128, 1])
    IDT = din("ident", [128, 128], MMDT)
    OUT = nc.dram_tensor("out_fm", [C, 128, N], F32, kind="ExternalOutput").ap()

    ACCR = nc.dram_tensor("acc_r", [C, 128, N], F32).ap()
    ACCZ = nc.dram_tensor("acc_z", [C, 128, N], F32).ap()
    ACCN = nc.dram_tensor("acc_n", [C, 128, N], F32).ap()
    XRH = nc.dram_tensor("xrh_nm_d", [nt, 128, BF], MMDT).ap()

    with tile.TileContext(nc) as tc:
        with (
            tc.tile_pool(name="nm", bufs=32) as nm_pool,
            tc.tile_pool(name="fm", bufs=12) as fm_pool,
            tc.tile_pool(name="gate", bufs=4) as gate_pool,
            tc.tile_pool(name="wt", bufs=6) as wt_pool,
            tc.tile_pool(name="wxi", bufs=18) as wxi_pool,
            tc.tile_pool(name="aux", bufs=12) as aux_pool,
            tc.tile_pool(name="stg", bufs=4) as stg_pool,
            tc.tile_pool(name="const", bufs=1) as const_pool,
            tc.tile_pool(name="ps", bufs=6, space="PSUM") as ps_pool,
            tc.tile_pool(name="psx", bufs=2, space="PSUM") as psx_pool,
        ):
            ident = const_pool.tile([128, 128], MMDT, tag="ident")
            nc.sync.dma_start(ident[:], IDT[:])
            brt = const_pool.tile([128, 1], F32, tag="brt")
            nc.sync.dma_start(brt[:], BR[:])
            bzt = const_pool.tile([128, 1], F32, tag="bzt")
            nc.sync.dma_start(bzt[:], BZ[:])
            bnt = const_pool.tile([128, 1], F32, tag="bnt")
            nc.sync.dma_start(bnt[:], BN[:])

            def load_nm(SRC):
                ts = []
                for jt in range(nt):
                    t = nm_pool.tile([128, BF], MMDT, name="nmt", tag="nm")
                    nc.sync.dma_start(t[:], SRC[jt])
                    ts.append(t)
                return ts

            def hop(src, WT):
                """One diffusion hop; returns fm tiles (6 x [128, N])."""
                fms = [fm_pool.tile([128, N], MMDT, name="fmt", tag="fm") for _ in range(6)]
                for ibk in range(nbk):
                    pss = [ps_pool.tile([128, 512], F32, name="pst", tag="ps")
                           for _ in range(6)]
                    for jt in range(nt):
                        wt = wt_pool.tile([128, 512], MMDT, name="wtt", tag="wt")
                        nc.sync.dma_start(
                            wt[:], WT[jt][:, 512 * ibk:512 * (ibk + 1)])
                        for c in range(6):
                            nc.tensor.matmul(
                                pss[c][:],
                                src[jt][:, 128 * c:128 * (c + 1)],
                                wt[:],
                                start=(jt == 0), stop=(jt == nt - 1))
                    for c in range(6):
                        nc.vector.tensor_copy(
                            fms[c][:, 512 * ibk:512 * (ibk + 1)], pss[c][:])
                return fms

            def aux_of(fms):
                """Base-0 copies of rows [64:128) of each fm tile (so every
                MLP contraction segment sits at partition 0 -> one PSUM
                accumulation group, no mixed tile_position)."""
                auxs = []
                for t in range(6):
                    a = aux_pool.tile([64, N], MMDT, name="auxt", tag="aux")
                    nc.gpsimd.dma_start(a[:], fms[t][64:128, :])
                    auxs.append(a)
                return auxs

            def mlp_feed(fms, auxs, kidx, gates, first):
                """gates: list of (WXI dram, ACC dram). Accumulate logits."""
                for WXI, ACCD in gates:
                    wx = []
                    for s in range(3):
                        w = wxi_pool.tile([64, 128], MMDT, name="wxit", tag="wxi")
                        nc.gpsimd.dma_start(w[:], WXI[kidx][s])
                        wx.append(w)
                    for b in range(C):
                        for nb in range(nbk):
                            nbs = slice(512 * nb, 512 * (nb + 1))
                            ps = psx_pool.tile([128, 512], F32, name="psxt", tag="psx")
                            for s in range(3):
                                t, off = divmod(b * FI + 64 * s, 128)
                                rhs = (fms[t][0:64, nbs] if off == 0
                                       else auxs[t][0:64, nbs])
                                nc.tensor.matmul(ps[:], wx[s][:], rhs,
                                                 start=(s == 0), stop=(s == 2))
                            stg = stg_pool.tile([128, 512], F32, name="stgt", tag="stg")
                            nc.vector.tensor_copy(stg[:], ps[:])
                            nc.gpsimd.dma_start(
                                ACCD[b][:, nbs], stg[:],
                                accum_op=(ALU.bypass if first else ALU.add))

            def retranspose(fms):
                """fm tiles -> fresh nm tiles via PE transposes."""
                nms = [nm_pool.tile([128, BF], MMDT, name="nmt", tag="nm")
                       for _ in range(nt)]
                for it in range(nt):
                    ps = psx_pool.tile([128, BF], MMDT, name="psxt", tag="psx")
                    for c in range(6):
                        nc.tensor.transpose(
                            ps[:, 128 * c:128 * (c + 1)],
                            fms[c][:, 128 * it:128 * (it + 1)],
                            ident[:])
                    nc.vector.tensor_copy(nms[it][:], ps[:])
                return nms

            def diffusion(x_nm_loader, x_fm_tiles, gates, xnm_first=None):
                """Full 2-direction diffusion + MLP accumulation.
                MLP feeds are deferred one hop so they never gate the next
                hop's matmul stream (fm pool holds 2 chunks)."""
                mlp_feed(x_fm_tiles, aux_of(x_fm_tiles), 0, gates,
                         first=True)
                pending = None
                cur = xnm_first if xnm_first is not None else x_nm_loader()
                for wdir, WT in ((0, WFT), (1, WBT)):
                    if wdir == 1:
                        cur = x_nm_loader()
                    for k in range(1, NHOPS + 1):
                        fm = hop(cur, WT)
                        aux = aux_of(fm)
                        cur = retranspose(fm) if k < NHOPS else None
                        if pending is not None:
                            mlp_feed(*pending)
                        pending = (fm, aux, wdir * NHOPS + k, gates, False)
                mlp_feed(*pending)

            # ---------------- diffusion 1 (r, z gates) ----------------
            fm0 = []
            for t in range(6):
                f = fm_pool.tile([128, N], MMDT, name="fmt", tag="fm")
                nc.scalar.dma_start(f[:], XHFM[t])
                fm0.append(f)
            diffusion(lambda: load_nm(XH), fm0, [(WRI, ACCR), (WZI, ACCZ)])

            # ------------- gates r, z; assemble xrh (nm + fm) -------------
            xrh_nm = [nm_pool.tile([128, BF], MMDT, name="nmt", tag="nm")
                      for _ in range(nt)]
            xrh_fm = [fm_pool.tile([128, N], MMDT, name="fmt", tag="fm") for _ in range(6)]
            for b in range(C):
                accr = gate_pool.tile([128, N], F32, name="gatet", tag="gate")
                nc.scalar.dma_start(accr[:], ACCR[b])
                r = gate_pool.tile([128, N], F32, name="gatet", tag="gate")
                nc.scalar.activation(r[:], accr[:], AF.Sigmoid, bias=brt[:])
                h = gate_pool.tile([128, N], F32, name="gatet", tag="gate")
                nc.scalar.dma_start(h[:], HFM[b])
                rh = fm_pool.tile([128, N], MMDT, name="fmt", tag="fm")
                nc.vector.tensor_mul(rh[:], r[:], h[:])
                # rh columns of xrh_nm (PE transpose 128-blocks)
                for g in range(nt // 4):
                    ps = psx_pool.tile([128, 512], MMDT, name="psxt", tag="psx")
                    for q in range(4):
                        it = 4 * g + q
                        nc.tensor.transpose(
                            ps[:, 128 * q:128 * (q + 1)],
                            rh[:, 128 * it:128 * (it + 1)], ident[:])
                    for q in range(4):
                        nc.vector.tensor_copy(
                            xrh_nm[4 * g + q][:, b * FI + 64:(b + 1) * FI],
                            ps[:, 128 * q:128 * (q + 1)])
                # fm rows of xrh: x piece then two rh 64-row pieces
                t, off = divmod(b * FI, 128)
                nc.scalar.dma_start(xrh_fm[t][off:off + 64, :], XFM[b])
                for s2 in range(2):
                    t, off = divmod(b * FI + 64 + 64 * s2, 128)
                    nc.scalar.dma_start(xrh_fm[t][off:off + 64, :],
                                        rh[64 * s2:64 * (s2 + 1), :])
            # x columns of xrh_nm straight from the xh param
            for jt in range(nt):
                for b in range(C):
                    nc.scalar.dma_start(xrh_nm[jt][:, b * FI:b * FI + 64],
                                        XH[jt][:, b * FI:b * FI + 64])
            # spill xrh_nm for the backward-chain reload
            for jt in range(nt):
                nc.sync.dma_start(XRH[jt], xrh_nm[jt][:])

            # ---------------- diffusion 2 (n gate) ----------------
            diffusion(lambda: load_nm(XRH), xrh_fm, [(WNI, ACCN)],
                      xnm_first=xrh_nm)

            # ---------------- final gate ----------------
            for b in range(C):
                accn = gate_pool.tile([128, N], F32, name="gatet", tag="gate")
                nc.scalar.dma_start(accn[:], ACCN[b])
                n_t = gate_pool.tile([128, N], F32, name="gatet", tag="gate")
                nc.scalar.activation(n_t[:], accn[:], AF.Tanh, bias=bnt[:])
                h = gate_pool.tile([128, N], F32, name="gatet", tag="gate")
                nc.scalar.dma_start(h[:], HFM[b])
                accz = gate_pool.tile([128, N], F32, name="gatet", tag="gate")
                nc.scalar.dma_start(accz[:], ACCZ[b])
                z = gate_pool.tile([128, N], F32, name="gatet", tag="gate")
                nc.scalar.activation(z[:], accz[:], AF.Sigmoid, bias=bzt[:])
                d = gate_pool.tile([128, N], F32, name="gatet", tag="gate")
                nc.vector.tensor_sub(d[:], n_t[:], h[:])
                zd2 = gate_pool.tile([128, N], F32, name="gatet", tag="gate")
                nc.vector.tensor_mul(zd2[:], z[:], d[:])
                o = gate_pool.tile([128, N], F32, name="gatet", tag="gate")
                nc.vector.tensor_add(o[:], zd2[:], h[:])
                nc.scalar.dma_start(OUT[b], o[:])

    nc.compile()
    return nc


def _pack_interleaved(W):
    """[128, 7*192] torch-Linear weight -> [7, 3, 64, 128] transposed 64-row
    contraction segments: out[k, s, f, o] = W[o, k*192 + 64*s + f]."""
    out = np.zeros((7, 3, 64, 128), np.float32)
    for k in range(7):
        for s in range(3):
            out[k, s] = W[:, k * FI + 64 * s:k * FI + 64 * (s + 1)].T
    return np.ascontiguousarray(out)


_NC_CACHE = {}


def _get_nc(nt):
    if nt not in _NC_CACHE:
        _NC_CACHE[nt] = build_nc(nt)
    return _NC_CACHE[nt]


def make_in_maps(x, h_prev, W_fwd, W_bwd, Wr, br, Wz, bz, Wn, bn):
    mdt = np.dtype(ml_dtypes.bfloat16) if MM_BF16 else np.float32
    x = np.asarray(x, np.float32)
    h_prev = np.asarray(h_prev, np.float32)
    B, N, Din = x.shape
    nt = N // 128
    WfT = np.ascontiguousarray(np.asarray(W_fwd, np.float32).T).reshape(nt, 128, N)
    WbT = np.ascontiguousarray(np.asarray(W_bwd, np.float32).T).reshape(nt, 128, N)
    wrI = _pack_interleaved(np.asarray(Wr, np.float32))
    wzI = _pack_interleaved(np.asarray(Wz, np.float32))
    wnI = _pack_interleaved(np.asarray(Wn, np.float32))
    ident = np.ascontiguousarray(np.eye(128, dtype=np.float32))
    WfT_d = WfT.astype(mdt)
    WbT_d = WbT.astype(mdt)
    wrI_d = wrI.astype(mdt)
    wzI_d = wzI.astype(mdt)
    wnI_d = wnI.astype(mdt)
    ident_d = ident.astype(mdt)
    brc = np.ascontiguousarray(np.asarray(br, np.float32).reshape(128, 1))
    bzc = np.ascontiguousarray(np.asarray(bz, np.float32).reshape(128, 1))
    bnc = np.ascontiguousarray(np.asarray(bn, np.float32).reshape(128, 1))
    ncores = B // C
    in_maps = []
    for cix in range(ncores):
        xs = x[C * cix:C * (cix + 1)]
        hs = h_prev[C * cix:C * (cix + 1)]
        xh = np.concatenate([xs, hs], axis=-1)            # [C, N, 192]
        flat = np.ascontiguousarray(xh.transpose(1, 0, 2).reshape(N, BF))
        xh_nm = np.ascontiguousarray(flat).reshape(nt, 128, BF)
        xh_fm = np.ascontiguousarray(flat.T).reshape(6, 128, N)
        x_fm = np.ascontiguousarray(xs.transpose(0, 2, 1))
        h_fm = np.ascontiguousarray(hs.transpose(0, 2, 1))
        in_maps.append(dict(
            xh_nm=xh_nm.astype(mdt), xh_fm=xh_fm.astype(mdt),
            wfT=WfT_d, wbT=WbT_d, wrI=wrI_d, wzI=wzI_d, wnI=wnI_d,
            x_fm=x_fm.astype(mdt), h_fm=h_fm,
            br_c=brc, bz_c=bzc, bn_c=bnc, ident=ident_d))
    return in_maps, nt, ncores


def kernel(x, h_prev, W_fwd, W_bwd, Wr, br, Wz, bz, Wn, bn, _trace=False):
    in_maps, nt, ncores = make_in_maps(
        x, h_prev, W_fwd, W_bwd, Wr, br, Wz, bz, Wn, bn)
    nc = _get_nc(nt)
    res = run_bass_kernel_spmd(nc, in_maps, list(range(ncores)), trace=_trace)
    outs = [np.ascontiguousarray(res.results[c]["out_fm"].transpose(0, 2, 1))
            for c in range(ncores)]
    full = np.concatenate(outs, axis=0).astype(np.float32)
    if _trace:
        return full, res
    return full



# revision 8
# speedup vs baseline: 2.4281x; 2.4281x over previous
"""DCGRU cell on 8 Trainium2 NeuronCores — fp8 DoubleRow edition.

Sharding: data-parallel over batch (B=32 -> 4 per core), adjacency + MLP
weights replicated. No collectives; host gathers per-core outputs.

Key ideas vs the bf16 baseline:
  * Diffusion hop matmuls run in fp8e4 with MatmulPerfMode.DoubleRow: each
    instruction contracts TWO 128-row k-tiles (lhsT [128,2,M], rhs [128,2,N])
    at 0.5 cycles/output-col — 2-4x the bf16 rate.
  * Diffusion 2 only propagates the r*h feature columns (128/batch instead of
    192): the x-part hop features are identical to diffusion 1's and are
    reused for the n-gate MLP. Saves 1/3 of diffusion-2 hop FLOPs.
  * Gate logits accumulate across all 7 k-blocks in a single PSUM group per
    (batch, 512-col block) — no DRAM accumulators, no accum DMAs. Hop
    features spill to DRAM (fp8) and are gathered back per block in paired
    DoubleRow layout.
  * The k=0 MLP segments (the raw x_h / rh features, which dominate logit
    magnitude) stay bf16 with weights pre-scaled by 8192 so they share the
    PSUM accumulation group with the fp8 hop segments.

Scaling scheme (fp8e4 max normal 240):
  x_h, rh stored *16; W stored *512; hop feats stored *128.
  hop1 psum = 16*512*hop  -> copy scale 1/64  -> *128
  hopk psum = 128*512*hop -> copy scale 1/512 -> *128
  MLP hop weights *64 -> logit psum = 128*64 = 8192*logit
  k0 weights: *8192 (vs raw x_h bf16), *512 (vs rh16 bf16)
  activation scale 1/8192 recovers logits.

Per-batch feature order matches the reference concat:
  k-blocks [x_h, Wf^1, Wf^2, Wf^3, Wb^1, Wb^2, Wb^3], 192 feats each.
"""

import sys
import numpy as np
import ml_dtypes

for _p in ("/opt/trn_rl_repo",):
    if _p not in sys.path:
        sys.path.insert(0, _p)

from concourse import bacc, tile, mybir  # noqa: E402
from concourse.bass_utils import run_bass_kernel_spmd  # noqa: E402

F32 = mybir.dt.float32
BF16 = mybir.dt.bfloat16
FP8 = mybir.dt.float8e4
AF = mybir.ActivationFunctionType
DR = mybir.MatmulPerfMode.DoubleRow
E4 = ml_dtypes.float8_e4m3
BF = ml_dtypes.bfloat16

C = 4            # batches per core
FI = 192         # per-batch feature width in d1 (x 64 + h 128)
DH = 128
NCORES = 8
NHOPS = 3
NJP = 8          # node-tile pairs (N = NJP*256)
NBK = 4          # 512-col node blocks
SX = 16.0        # x_h / rh fp8 scale
SW = 512.0       # W fp8 scale
SF = 128.0       # hop-feature fp8 scale
SMW = 64.0       # MLP hop-weight fp8 scale
SPS = SF * SMW   # logit psum scale (8192)


def build_nc():
    N = NJP * 256
    nc = bacc.Bacc("TRN2", target_bir_lowering=False, debug=False,
                   num_devices=NCORES)

    def din(name, shape, dt=F32):
        return nc.dram_tensor(name, shape, dt, kind="ExternalInput").ap()

    XHNM = din("xh_nm8", [NJP, 128, 2, 768], FP8)     # 16*x_h node-major paired
    XHK0 = din("xh_k0", [C, FI, N], BF16)             # x_h feature-major raw
    WFP = din("wfp", [NJP, 128, 2, N], FP8)           # 512*W_fwd^T paired
    WBP = din("wbp", [NJP, 128, 2, N], FP8)
    HFM = din("h_fm", [C, DH, N])                     # h_prev feature-major f32
    # MLP weights (see packer)
    WK0 = {g: din(f"w{g}k0", [FI, DH], BF16) for g in ("r", "z")}
    WA = {g: din(f"w{g}A", [3, 128, 2, DH], FP8) for g in ("r", "z")}
    WB = {g: din(f"w{g}B", [3, 64, 2, DH], FP8) for g in ("r", "z")}
    WNK0X = din("wnk0x", [64, DH], BF16)
    WNK0R = din("wnk0r", [DH, DH], BF16)
    WNX = din("wnx", [3, 64, 2, DH], FP8)
    WNR = din("wnr", [3, 128, 2, DH], FP8)
    BRT = din("br_c", [DH, 1])
    BZT = din("bz_c", [DH, 1])
    BNT = din("bn_c", [DH, 1])
    IDB = din("identb", [128, 128], BF16)
    OUT = nc.dram_tensor("out_fm", [C, DH, N], F32, kind="ExternalOutput").ap()

    FEAT1 = nc.dram_tensor("feat1", [6, 768, N], FP8).ap()   # d1 feats *128
    FEAT2 = nc.dram_tensor("feat2", [6, C * DH, N], FP8).ap()  # d2 rh feats *128

    with tile.TileContext(nc) as tc:
        with (
            tc.tile_pool(name="const", bufs=1) as cpool,
            tc.tile_pool(name="nmx", bufs=8) as nmx_pool,
            tc.tile_pool(name="nm1", bufs=12) as nm1_pool,
            tc.tile_pool(name="nm2", bufs=12) as nm2_pool,
            tc.tile_pool(name="wc", bufs=8) as wc_pool,
            tc.tile_pool(name="fm1", bufs=6) as fm1_pool,
            tc.tile_pool(name="fm2", bufs=4) as fm2_pool,
            tc.tile_pool(name="stg", bufs=12) as stg_pool,
            tc.tile_pool(name="feed", bufs=8) as feed_pool,
            tc.tile_pool(name="k0p", bufs=4) as k0_pool,
            tc.tile_pool(name="gw", bufs=2) as gw_pool,
            tc.tile_pool(name="gres", bufs=4) as gres_pool,
            tc.tile_pool(name="ps", bufs=6, space="PSUM") as ps_pool,
            tc.tile_pool(name="pst", bufs=2, space="PSUM") as pst_pool,
        ):
            # ---------------- constants ----------------
            identb = cpool.tile([128, 128], BF16, tag="idb")
            nc.sync.dma_start(identb[:], IDB[:])
            brt = cpool.tile([DH, 1], F32, tag="brt")
            nc.sync.dma_start(brt[:], BRT[:])
            bzt = cpool.tile([DH, 1], F32, tag="bzt")
            nc.sync.dma_start(bzt[:], BZT[:])
            bnt = cpool.tile([DH, 1], F32, tag="bnt")
            nc.sync.dma_start(bnt[:], BNT[:])
            wk0a, wk0b, wa, wb = {}, {}, {}, {}
            for g in ("r", "z"):
                wk0a[g] = cpool.tile([128, DH], BF16, name="t", tag=f"w{g}k0a")
                nc.sync.dma_start(wk0a[g][:], WK0[g][0:128, :])
                wk0b[g] = cpool.tile([64, DH], BF16, name="t", tag=f"w{g}k0b")
                nc.sync.dma_start(wk0b[g][:], WK0[g][128:192, :])
                wa[g] = []
                wb[g] = []
                for p in range(3):
                    t = cpool.tile([128, 2, DH], FP8, name="t", tag=f"w{g}A{p}")
                    nc.sync.dma_start(t[:], WA[g][p])
                    wa[g].append(t)
                    t = cpool.tile([64, 2, DH], FP8, name="t", tag=f"w{g}B{p}")
                    nc.sync.dma_start(t[:], WB[g][p])
                    wb[g].append(t)
            wnk0x = cpool.tile([64, DH], BF16, tag="wnk0x")
            nc.sync.dma_start(wnk0x[:], WNK0X[:])
            wnk0r = cpool.tile([DH, DH], BF16, tag="wnk0r")
            nc.sync.dma_start(wnk0r[:], WNK0R[:])
            wnx, wnr = [], []
            for p in range(3):
                t = cpool.tile([64, 2, DH], FP8, name="t", tag=f"wnx{p}")
                nc.sync.dma_start(t[:], WNX[p])
                wnx.append(t)
                t = cpool.tile([128, 2, DH], FP8, name="t", tag=f"wnr{p}")
                nc.sync.dma_start(t[:], WNR[p])
                wnr.append(t)

            # resident paired node-major x_h (chain start for both dirs)
            nm_xh = []
            for jp in range(NJP):
                t = nmx_pool.tile([128, 2, 768], FP8, name="t", tag="nmx")
                nc.sync.dma_start(t[:], XHNM[jp])
                nm_xh.append(t)

            def load_wdir(WP):
                ws = []
                for jp in range(NJP):
                    t = wc_pool.tile([128, 2, N], FP8, name="t", tag="w")
                    nc.sync.dma_start(t[:], WP[jp])
                    ws.append(t)
                return ws

            def hop(cur, ws, nch, k, FEATD, rowbase):
                """One DoubleRow hop. cur: paired nm tiles [128,2,128*nch].
                Spills fp8 *SF feats to FEATD; returns bf16 fm tiles for
                chain re-entry (None on the last hop)."""
                fm16s = None
                if k < NHOPS:
                    pool = fm1_pool if nch == 6 else fm2_pool
                    tg = "fm1" if nch == 6 else "fm2"
                    fm16s = [pool.tile([128, N], BF16, name="t", tag=tg)
                             for _ in range(nch)]
                scale = 1.0 / 64.0 if k == 1 else 1.0 / 512.0
                for blk in range(NBK):
                    cs = slice(512 * blk, 512 * (blk + 1))
                    pss = [ps_pool.tile([128, 512], F32, name="t", tag="hop")
                           for _ in range(nch)]
                    for jp in range(NJP):
                        rhs = ws[jp][:, :, cs]
                        for c in range(nch):
                            nc.tensor.matmul(
                                pss[c][:],
                                cur[jp][:, :, 128 * c:128 * (c + 1)],
                                rhs,
                                start=(jp == 0), stop=(jp == NJP - 1),
                                perf_mode=DR)
                    for c in range(nch):
                        stg = stg_pool.tile([128, 512], FP8, name="t",
                                            tag="stg")
                        nc.scalar.activation(stg[:], pss[c][:], AF.Copy,
                                             scale=scale)
                        nc.scalar.dma_start(
                            FEATD[rowbase + 128 * c:rowbase + 128 * (c + 1),
                                  cs], stg[:])
                        if fm16s is not None:
                            nc.vector.tensor_scalar_mul(
                                fm16s[c][:, cs], pss[c][:], scale)
                return fm16s

            def retranspose(fms, nch):
                """bf16 fm tiles -> fresh paired fp8 nm tiles (PE transpose
                in bf16, psum->sbuf copy casts to fp8)."""
                nms = []
                for jp in range(NJP):
                    t = (nm1_pool.tile([128, 2, 768], FP8, name="t", tag="nm1")
                         if nch == 6 else
                         nm2_pool.tile([128, 2, 512], FP8, name="t", tag="nm2"))
                    for h in range(2):
                        it = 2 * jp + h
                        ps = pst_pool.tile([128, 128 * nch], BF16, name="t",
                                           tag="tr")
                        for c in range(nch):
                            nc.tensor.transpose(
                                ps[:, 128 * c:128 * (c + 1)],
                                fms[c][:, 128 * it:128 * (it + 1)],
                                identb[:])
                        nc.vector.tensor_copy(t[:, h, :], ps[:])
                    nms.append(t)
                return nms

            # ---------------- diffusion 1 ----------------
            with nc.named_scope("d1_hops"):
                for dirw, WP in ((0, WFP), (1, WBP)):
                    ws = load_wdir(WP)
                    cur = nm_xh
                    for k in range(1, NHOPS + 1):
                        kidx = dirw * NHOPS + k  # 1..6
                        fms = hop(cur, ws, 6, k, FEAT1[kidx - 1], 0)
                        if k < NHOPS:
                            cur = retranspose(fms, 6)

            # ---------------- MLP r,z + rh ----------------
            sc_rz = nc.enter_named_scope("mlp_rz", False)
            z16 = [gres_pool.tile([DH, N], BF16, name="t", tag="z16")
                   for _ in range(C)]
            rh16 = [gres_pool.tile([DH, N], BF16, name="t", tag="rh16")
                    for _ in range(C)]
            for b in range(C):
                for blk in range(NBK):
                    cs = slice(512 * blk, 512 * (blk + 1))
                    k0a = k0_pool.tile([128, 512], BF16, name="t", tag="k0a")
                    nc.gpsimd.dma_start(k0a[:], XHK0[b][0:128, cs])
                    k0b = k0_pool.tile([64, 512], BF16, name="t", tag="k0b")
                    nc.gpsimd.dma_start(k0b[:], XHK0[b][128:192, cs])
                    fA, fB = [], []
                    for p in range(3):
                        ka, kb = 2 * p + 1, 2 * p + 2
                        tA = feed_pool.tile([128, 2, 512], FP8, name="t",
                                            tag="fA")
                        nc.gpsimd.dma_start(
                            tA[:, 0, :], FEAT1[ka - 1][b * FI:b * FI + 128, cs])
                        nc.gpsimd.dma_start(
                            tA[:, 1, :], FEAT1[kb - 1][b * FI:b * FI + 128, cs])
                        fA.append(tA)
                        tB = feed_pool.tile([64, 2, 512], FP8, name="t",
                                            tag="fB")
                        nc.gpsimd.dma_start(
                            tB[:, 0, :],
                            FEAT1[ka - 1][b * FI + 128:b * FI + 192, cs])
                        nc.gpsimd.dma_start(
                            tB[:, 1, :],
                            FEAT1[kb - 1][b * FI + 128:b * FI + 192, cs])
                        fB.append(tB)
                    hblk = gw_pool.tile([DH, 512], F32, name="t", tag="h")
                    nc.sync.dma_start(hblk[:], HFM[b][:, cs])
                    for g in ("r", "z"):
                        ps = ps_pool.tile([128, 512], F32, name="t", tag="hop")
                        nc.tensor.matmul(ps[:], wk0a[g][:], k0a[:],
                                         start=True, stop=False)
                        nc.tensor.matmul(ps[:], wk0b[g][:], k0b[:],
                                         start=False, stop=False)
                        for p in range(3):
                            nc.tensor.matmul(ps[:], wa[g][p][:], fA[p][:],
                                             start=False, stop=False,
                                             perf_mode=DR)
                            nc.tensor.matmul(ps[:], wb[g][p][:], fB[p][:],
                                             start=False, stop=(p == 2),
                                             perf_mode=DR)
                        if g == "r":
                            rwk = gw_pool.tile([DH, 512], F32, name="t",
                                               tag="rw")
                            nc.scalar.activation(rwk[:], ps[:], AF.Sigmoid,
                                                 bias=brt[:], scale=1.0 / SPS)
                            nc.vector.scalar_tensor_tensor(
                                rh16[b][:, cs], rwk[:], SX, hblk[:],
                                mybir.AluOpType.mult, mybir.AluOpType.mult)
                        else:
                            nc.scalar.activation(z16[b][:, cs], ps[:],
                                                 AF.Sigmoid, bias=bzt[:],
                                                 scale=1.0 / SPS)

            nc.leave_named_scope("mlp_rz", sc_rz[0], False)

            # ---------------- diffusion 2 (rh chain) ----------------
            def build_nm2():
                nms = []
                for jp in range(NJP):
                    ps = pst_pool.tile([128, 2, 512], BF16, name="t", tag="tr")
                    for h in range(2):
                        it = 2 * jp + h
                        for b in range(C):
                            nc.tensor.transpose(
                                ps[:, h, 128 * b:128 * (b + 1)],
                                rh16[b][:, 128 * it:128 * (it + 1)],
                                identb[:])
                    t = nm2_pool.tile([128, 2, 512], FP8, name="t", tag="nm2")
                    nc.vector.tensor_copy(t[:], ps[:])
                    nms.append(t)
                return nms

            with nc.named_scope("d2_hops"):
                for dirw, WP in ((0, WFP), (1, WBP)):
                    ws = load_wdir(WP)
                    cur = build_nm2()
                    for k in range(1, NHOPS + 1):
                        kidx = dirw * NHOPS + k
                        fms = hop(cur, ws, 4, k, FEAT2[kidx - 1], 0)
                        if k < NHOPS:
                            cur = retranspose(fms, 4)

            # ---------------- MLP n + final gate ----------------
            sc_n = nc.enter_named_scope("mlp_n", False)
            for b in range(C):
                for blk in range(NBK):
                    cs = slice(512 * blk, 512 * (blk + 1))
                    k0x = k0_pool.tile([64, 512], BF16, name="t", tag="k0b")
                    nc.gpsimd.dma_start(k0x[:], XHK0[b][0:64, cs])
                    fx, fr = [], []
                    for p in range(3):
                        ka, kb = 2 * p + 1, 2 * p + 2
                        tX = feed_pool.tile([64, 2, 512], FP8, name="t",
                                            tag="fB")
                        nc.gpsimd.dma_start(
                            tX[:, 0, :], FEAT1[ka - 1][b * FI:b * FI + 64, cs])
                        nc.gpsimd.dma_start(
                            tX[:, 1, :], FEAT1[kb - 1][b * FI:b * FI + 64, cs])
                        fx.append(tX)
                        tR = feed_pool.tile([128, 2, 512], FP8, name="t",
                                            tag="fA")
                        nc.gpsimd.dma_start(
                            tR[:, 0, :], FEAT2[ka - 1][b * DH:b * DH + 128, cs])
                        nc.gpsimd.dma_start(
                            tR[:, 1, :], FEAT2[kb - 1][b * DH:b * DH + 128, cs])
                        fr.append(tR)
                    hblk = gw_pool.tile([DH, 512], F32, name="t", tag="h")
                    nc.sync.dma_start(hblk[:], HFM[b][:, cs])
                    ps = ps_pool.tile([128, 512], F32, name="t", tag="hop")
                    nc.tensor.matmul(ps[:], wnk0x[:], k0x[:],
                                     start=True, stop=False)
                    nc.tensor.matmul(ps[:], wnk0r[:], rh16[b][:, cs],
                                     start=False, stop=False)
                    for p in range(3):
                        nc.tensor.matmul(ps[:], wnx[p][:], fx[p][:],
                                         start=False, stop=False, perf_mode=DR)
                        nc.tensor.matmul(ps[:], wnr[p][:], fr[p][:],
                                         start=False, stop=(p == 2),
                                         perf_mode=DR)
                    nf = gw_pool.tile([DH, 512], F32, name="t", tag="nf")
                    nc.scalar.activation(nf[:], ps[:], AF.Tanh, bias=bnt[:],
                                         scale=1.0 / SPS)
                    dlt = gw_pool.tile([DH, 512], F32, name="t", tag="dw")
                    nc.vector.tensor_sub(dlt[:], nf[:], hblk[:])
                    zd = gw_pool.tile([DH, 512], F32, name="t", tag="zd")
                    nc.vector.tensor_mul(zd[:], z16[b][:, cs], dlt[:])
                    o = gw_pool.tile([DH, 512], F32, name="t", tag="o")
                    nc.vector.tensor_add(o[:], zd[:], hblk[:])
                    nc.scalar.dma_start(OUT[b][:, cs], o[:])
            nc.leave_named_scope("mlp_n", sc_n[0], False)

    nc.compile()
    return nc


_NC_CACHE = {}


def _get_nc():
    if "nc" not in _NC_CACHE:
        _NC_CACHE["nc"] = build_nc()
    return _NC_CACHE["nc"]


def _pack_gate_w(W):
    """W [128, 1344] -> (wk0 bf16 [192,128]*8192, wA fp8 [3,128,2,128]*64,
    wB fp8 [3,64,2,128]*64)."""
    W = np.asarray(W, np.float32)
    wk0 = np.ascontiguousarray((W[:, 0:FI].T * SPS)).astype(BF)
    wA = np.zeros((3, 128, 2, DH), np.float32)
    wBt = np.zeros((3, 64, 2, DH), np.float32)
    for p in range(3):
        for h in range(2):
            k = 2 * p + 1 + h
            blkc = W[:, k * FI:(k + 1) * FI]          # [128, 192]
            wA[p, :, h, :] = blkc[:, 0:128].T * SMW
            wBt[p, :, h, :] = blkc[:, 128:192].T * SMW
    return wk0, wA.astype(E4), wBt.astype(E4)


def _pack_n_w(W):
    """Wn [128, 1344] -> k0x bf16 [64,128]*8192, k0r bf16 [128,128]*512,
    wnx fp8 [3,64,2,128]*64, wnr fp8 [3,128,2,128]*64."""
    W = np.asarray(W, np.float32)
    k0x = np.ascontiguousarray(W[:, 0:64].T * SPS).astype(BF)
    k0r = np.ascontiguousarray(W[:, 64:FI].T * (SPS / SX)).astype(BF)
    wnx = np.zeros((3, 64, 2, DH), np.float32)
    wnr = np.zeros((3, 128, 2, DH), np.float32)
    for p in range(3):
        for h in range(2):
            k = 2 * p + 1 + h
            blkc = W[:, k * FI:(k + 1) * FI]
            wnx[p, :, h, :] = blkc[:, 0:64].T * SMW
            wnr[p, :, h, :] = blkc[:, 64:FI].T * SMW
    return k0x, k0r, wnx.astype(E4), wnr.astype(E4)


def _pack_wpair(W):
    """W [N,N] -> fp8 [NJP,128,2,N]: [jp,p,h,i] = 512*W[i, jp*256+h*128+p]."""
    WT = np.asarray(W, np.float32).T * SW                 # [j, i]
    N = WT.shape[0]
    return np.ascontiguousarray(
        WT.reshape(NJP, 2, 128, N).transpose(0, 2, 1, 3)).astype(E4)


def make_in_maps(x, h_prev, W_fwd, W_bwd, Wr, br, Wz, bz, Wn, bn):
    x = np.asarray(x, np.float32)
    h_prev = np.asarray(h_prev, np.float32)
    B, N, Din = x.shape
    wfp = _pack_wpair(W_fwd)
    wbp = _pack_wpair(W_bwd)
    wrk0, wrA, wrB = _pack_gate_w(Wr)
    wzk0, wzA, wzB = _pack_gate_w(Wz)
    wnk0x, wnk0r, wnx, wnr = _pack_n_w(Wn)
    identb = np.eye(128, dtype=np.float32).astype(BF)
    brc = np.ascontiguousarray(np.asarray(br, np.float32).reshape(DH, 1))
    bzc = np.ascontiguousarray(np.asarray(bz, np.float32).reshape(DH, 1))
    bnc = np.ascontiguousarray(np.asarray(bn, np.float32).reshape(DH, 1))
    ncores = B // C
    in_maps = []
    for cix in range(ncores):
        xs = x[C * cix:C * (cix + 1)]
        hs = h_prev[C * cix:C * (cix + 1)]
        xh = np.concatenate([xs, hs], axis=-1)            # [C, N, 192]
        flat = np.ascontiguousarray(xh.transpose(1, 0, 2).reshape(N, C * FI))
        xh_nm8 = np.ascontiguousarray(
            (flat * SX).reshape(NJP, 2, 128, C * FI).transpose(0, 2, 1, 3)
        ).astype(E4)
        xh_k0 = np.ascontiguousarray(xh.transpose(0, 2, 1)).astype(BF)
        h_fm = np.ascontiguousarray(hs.transpose(0, 2, 1))
        in_maps.append(dict(
            xh_nm8=xh_nm8, xh_k0=xh_k0, wfp=wfp, wbp=wbp, h_fm=h_fm,
            wrk0=wrk0, wrA=wrA, wrB=wrB, wzk0=wzk0, wzA=wzA, wzB=wzB,
            wnk0x=wnk0x, wnk0r=wnk0r, wnx=wnx, wnr=wnr,
            br_c=brc, bz_c=bzc, bn_c=bnc, identb=identb))
    return in_maps, ncores


def kernel(x, h_prev, W_fwd, W_bwd, Wr, br, Wz, bz, Wn, bn, _trace=False):
    in_maps, ncores = make_in_maps(
        x, h_prev, W_fwd, W_bwd, Wr, br, Wz, bz, Wn, bn)
    nc = _get_nc()
    res = run_bass_kernel_spmd(nc, in_maps, list(range(ncores)), trace=_trace)
    outs = [np.ascontiguousarray(res.results[c]["out_fm"].transpose(0, 2, 1))
            for c in range(ncores)]
    full = np.concatenate(outs, axis=0).astype(np.float32)
    if _trace:
        return full, res
    return full


# revision 11
# speedup vs baseline: 2.4896x; 1.0253x over previous
"""DCGRU cell on 8 Trainium2 NeuronCores — fp8 DoubleRow edition.

Sharding: data-parallel over batch (B=32 -> 4 per core), adjacency + MLP
weights replicated. No collectives; host gathers per-core outputs.

Key ideas vs the bf16 baseline:
  * Diffusion hop matmuls run in fp8e4 with MatmulPerfMode.DoubleRow: each
    instruction contracts TWO 128-row k-tiles (lhsT [128,2,M], rhs [128,2,N])
    at 0.5 cycles/output-col — 2-4x the bf16 rate.
  * Diffusion 2 only propagates the r*h feature columns (128/batch instead of
    192): the x-part hop features are identical to diffusion 1's and are
    reused for the n-gate MLP. Saves 1/3 of diffusion-2 hop FLOPs.
  * Gate logits accumulate across all 7 k-blocks in a single PSUM group per
    (batch, 512-col block) — no DRAM accumulators, no accum DMAs. Hop
    features spill to DRAM (fp8) and are gathered back per block in paired
    DoubleRow layout.
  * The k=0 MLP segments (the raw x_h / rh features, which dominate logit
    magnitude) stay bf16 with weights pre-scaled by 8192 so they share the
    PSUM accumulation group with the fp8 hop segments.

Scaling scheme (fp8e4 max normal 240):
  x_h, rh stored *16; W stored *512; hop feats stored *128.
  hop1 psum = 16*512*hop  -> copy scale 1/64  -> *128
  hopk psum = 128*512*hop -> copy scale 1/512 -> *128
  MLP hop weights *64 -> logit psum = 128*64 = 8192*logit
  k0 weights: *8192 (vs raw x_h bf16), *512 (vs rh16 bf16)
  activation scale 1/8192 recovers logits.

Per-batch feature order matches the reference concat:
  k-blocks [x_h, Wf^1, Wf^2, Wf^3, Wb^1, Wb^2, Wb^3], 192 feats each.
"""

import sys
import numpy as np
import ml_dtypes

for _p in ("/opt/trn_rl_repo",):
    if _p not in sys.path:
        sys.path.insert(0, _p)

from concourse import bacc, tile, mybir  # noqa: E402
from concourse.bass_utils import run_bass_kernel_spmd  # noqa: E402

F32 = mybir.dt.float32
BF16 = mybir.dt.bfloat16
FP8 = mybir.dt.float8e4
AF = mybir.ActivationFunctionType
DR = mybir.MatmulPerfMode.DoubleRow
E4 = ml_dtypes.float8_e4m3
BF = ml_dtypes.bfloat16

C = 4            # batches per core
FI = 192         # per-batch feature width in d1 (x 64 + h 128)
DH = 128
NCORES = 8
NHOPS = 3
NJP = 8          # node-tile pairs (N = NJP*256)
NBK = 4          # 512-col node blocks
SX = 16.0        # x_h / rh fp8 scale
SW = 512.0       # W fp8 scale
SF = 128.0       # hop-feature fp8 scale
SMW = 64.0       # MLP hop-weight fp8 scale
SPS = SF * SMW   # logit psum scale (8192)


def build_nc():
    N = NJP * 256
    nc = bacc.Bacc("TRN2", target_bir_lowering=False, debug=False,
                   num_devices=NCORES)

    def din(name, shape, dt=F32):
        return nc.dram_tensor(name, shape, dt, kind="ExternalInput").ap()

    XHNM = din("xh_nm8", [NJP, 128, 2, 768], FP8)     # 16*x_h node-major paired
    XHK0 = din("xh_k0", [C, FI, N], BF16)             # x_h feature-major raw
    WFP = din("wfp", [NJP, 128, 2, N], FP8)           # 512*W_fwd^T paired
    WBP = din("wbp", [NJP, 128, 2, N], FP8)
    HFM = din("h_fm", [C, DH, N])                     # h_prev feature-major f32
    # MLP weights (see packer)
    WK0 = {g: din(f"w{g}k0", [FI, DH], BF16) for g in ("r", "z")}
    WA = {g: din(f"w{g}A", [3, 128, 2, DH], FP8) for g in ("r", "z")}
    WB = {g: din(f"w{g}B", [3, 64, 2, DH], FP8) for g in ("r", "z")}
    WNK0X = din("wnk0x", [64, DH], BF16)
    WNK0R = din("wnk0r", [DH, DH], BF16)
    WNX = din("wnx", [3, 64, 2, DH], FP8)
    WNR = din("wnr", [3, 128, 2, DH], FP8)
    BRT = din("br_c", [DH, 1])
    BZT = din("bz_c", [DH, 1])
    BNT = din("bn_c", [DH, 1])
    IDB = din("identb", [128, 128], BF16)
    OUT = nc.dram_tensor("out_fm", [C, DH, N], F32, kind="ExternalOutput").ap()

    # k-pair-adjacent spill layouts: [pair][rows][h][N] so MLP gathers are
    # single 3-D DMAs
    FEAT1 = nc.dram_tensor("feat1", [3, 768, 2, N], FP8).ap()   # d1 feats *128
    FEAT2 = nc.dram_tensor("feat2", [3, C * DH, 2, N], FP8).ap()  # d2 feats

    with tile.TileContext(nc) as tc:
        with (
            tc.tile_pool(name="const", bufs=1) as cpool,
            tc.tile_pool(name="nmx", bufs=8) as nmx_pool,
            tc.tile_pool(name="nm1", bufs=10) as nm1_pool,
            tc.tile_pool(name="nm2", bufs=10) as nm2_pool,
            tc.tile_pool(name="wc", bufs=8) as wc_pool,
            tc.tile_pool(name="fm1", bufs=6) as fm1_pool,
            tc.tile_pool(name="fm2", bufs=4) as fm2_pool,
            tc.tile_pool(name="stg", bufs=9) as stg_pool,
            tc.tile_pool(name="feed", bufs=12) as feed_pool,
            tc.tile_pool(name="k0p", bufs=4) as k0_pool,
            tc.tile_pool(name="gw", bufs=2) as gw_pool,
            tc.tile_pool(name="gres", bufs=4) as gres_pool,
            tc.tile_pool(name="ps", bufs=6, space="PSUM") as ps_pool,
            tc.tile_pool(name="pst", bufs=2, space="PSUM") as pst_pool,
        ):
            # ---------------- constants ----------------
            identb = cpool.tile([128, 128], BF16, tag="idb")
            nc.scalar.dma_start(identb[:], IDB[:])
            brt = cpool.tile([DH, 1], F32, tag="brt")
            nc.scalar.dma_start(brt[:], BRT[:])
            bzt = cpool.tile([DH, 1], F32, tag="bzt")
            nc.scalar.dma_start(bzt[:], BZT[:])
            bnt = cpool.tile([DH, 1], F32, tag="bnt")
            nc.scalar.dma_start(bnt[:], BNT[:])
            wk0a, wk0b, wa, wb = {}, {}, {}, {}
            for g in ("r", "z"):
                wk0a[g] = cpool.tile([128, DH], BF16, name="t", tag=f"w{g}k0a")
                nc.scalar.dma_start(wk0a[g][:], WK0[g][0:128, :])
                wk0b[g] = cpool.tile([64, DH], BF16, name="t", tag=f"w{g}k0b")
                nc.scalar.dma_start(wk0b[g][:], WK0[g][128:192, :])
                wa[g] = []
                wb[g] = []
                for p in range(3):
                    t = cpool.tile([128, 2, DH], FP8, name="t", tag=f"w{g}A{p}")
                    nc.scalar.dma_start(t[:], WA[g][p])
                    wa[g].append(t)
                    t = cpool.tile([64, 2, DH], FP8, name="t", tag=f"w{g}B{p}")
                    nc.scalar.dma_start(t[:], WB[g][p])
                    wb[g].append(t)
            wnk0x = cpool.tile([64, DH], BF16, tag="wnk0x")
            nc.scalar.dma_start(wnk0x[:], WNK0X[:])
            wnk0r = cpool.tile([DH, DH], BF16, tag="wnk0r")
            nc.scalar.dma_start(wnk0r[:], WNK0R[:])
            wnx, wnr = [], []
            for p in range(3):
                t = cpool.tile([64, 2, DH], FP8, name="t", tag=f"wnx{p}")
                nc.scalar.dma_start(t[:], WNX[p])
                wnx.append(t)
                t = cpool.tile([128, 2, DH], FP8, name="t", tag=f"wnr{p}")
                nc.scalar.dma_start(t[:], WNR[p])
                wnr.append(t)

            # resident paired node-major x_h (chain start for both dirs)
            nm_xh = []
            for jp in range(NJP):
                t = nmx_pool.tile([128, 2, 768], FP8, name="t", tag="nmx")
                nc.gpsimd.dma_start(t[:], XHNM[jp])
                nm_xh.append(t)

            def load_wdir(WP):
                ws = []
                for jp in range(NJP):
                    t = wc_pool.tile([128, 2, N], FP8, name="t", tag="w")
                    nc.sync.dma_start(t[:], WP[jp])
                    ws.append(t)
                return ws

            def hop(cur, ws, nch, k, FEATD, kh, rowbase):
                """One DoubleRow hop. cur: paired nm tiles [128,2,128*nch].
                Spills fp8 *SF feats to FEATD; returns bf16 fm tiles for
                chain re-entry (None on the last hop)."""
                fm16s = None
                if k < NHOPS:
                    pool = fm1_pool if nch == 6 else fm2_pool
                    tg = "fm1" if nch == 6 else "fm2"
                    fm16s = [pool.tile([128, N], BF16, name="t", tag=tg)
                             for _ in range(nch)]
                scale = 1.0 / 64.0 if k == 1 else 1.0 / 512.0
                for blk in range(NBK):
                    cs = slice(512 * blk, 512 * (blk + 1))
                    pss = [ps_pool.tile([128, 512], F32, name="t", tag="hop")
                           for _ in range(nch)]
                    for jp in range(NJP):
                        rhs = ws[jp][:, :, cs]
                        for c in range(nch):
                            nc.tensor.matmul(
                                pss[c][:],
                                cur[jp][:, :, 128 * c:128 * (c + 1)],
                                rhs,
                                start=(jp == 0), stop=(jp == NJP - 1),
                                perf_mode=DR)
                    for c in range(nch):
                        stg = stg_pool.tile([128, 512], FP8, name="t",
                                            tag="stg")
                        nc.scalar.activation(stg[:], pss[c][:], AF.Copy,
                                             scale=scale)
                        nc.sync.dma_start(
                            FEATD[rowbase + 128 * c:rowbase + 128 * (c + 1),
                                  kh, cs], stg[:])
                        if fm16s is not None:
                            nc.vector.tensor_scalar_mul(
                                fm16s[c][:, cs], pss[c][:], scale)
                return fm16s

            def retranspose(fms, nch):
                """bf16 fm tiles -> fresh paired fp8 nm tiles (PE transpose
                in bf16, psum->sbuf copy casts to fp8)."""
                nms = []
                for jp in range(NJP):
                    t = (nm1_pool.tile([128, 2, 768], FP8, name="t", tag="nm1")
                         if nch == 6 else
                         nm2_pool.tile([128, 2, 512], FP8, name="t", tag="nm2"))
                    for h in range(2):
                        it = 2 * jp + h
                        ps = pst_pool.tile([128, 128 * nch], BF16, name="t",
                                           tag="tr")
                        for c in range(nch):
                            nc.tensor.transpose(
                                ps[:, 128 * c:128 * (c + 1)],
                                fms[c][:, 128 * it:128 * (it + 1)],
                                identb[:])
                        nc.vector.tensor_copy(t[:, h, :], ps[:])
                    nms.append(t)
                return nms

            # ---------------- diffusion 1 ----------------
            with nc.named_scope("d1_hops"):
                for dirw, WP in ((0, WFP), (1, WBP)):
                    ws = load_wdir(WP)
                    cur = nm_xh
                    for k in range(1, NHOPS + 1):
                        kidx = dirw * NHOPS + k  # 1..6
                        fms = hop(cur, ws, 6, k,
                                  FEAT1[(kidx - 1) // 2], (kidx - 1) % 2, 0)
                        if k < NHOPS:
                            cur = retranspose(fms, 6)

            # ---------------- MLP r,z + rh ----------------
            sc_rz = nc.enter_named_scope("mlp_rz", False)
            z16 = [gres_pool.tile([DH, N], BF16, name="t", tag="z16")
                   for _ in range(C)]
            rh16 = [gres_pool.tile([DH, N], BF16, name="t", tag="rh16")
                    for _ in range(C)]
            for b in range(C):
                for blk in range(NBK):
                    cs = slice(512 * blk, 512 * (blk + 1))
                    k0a = k0_pool.tile([128, 512], BF16, name="t", tag="k0a")
                    nc.gpsimd.dma_start(k0a[:], XHK0[b][0:128, cs])
                    k0b = k0_pool.tile([64, 512], BF16, name="t", tag="k0b")
                    nc.gpsimd.dma_start(k0b[:], XHK0[b][128:192, cs])
                    fA, fB = [], []
                    for p in range(3):
                        tA = feed_pool.tile([128, 2, 512], FP8, name="t",
                                            tag="fA")
                        nc.gpsimd.dma_start(
                            tA[:], FEAT1[p][b * FI:b * FI + 128, :, cs])
                        fA.append(tA)
                        tB = feed_pool.tile([64, 2, 512], FP8, name="t",
                                            tag="fB")
                        nc.scalar.dma_start(
                            tB[:], FEAT1[p][b * FI + 128:b * FI + 192, :, cs])
                        fB.append(tB)
                    hblk = gw_pool.tile([DH, 512], F32, name="t", tag="h")
                    nc.sync.dma_start(hblk[:], HFM[b][:, cs])
                    for g in ("r", "z"):
                        ps = ps_pool.tile([128, 512], F32, name="t", tag="hop")
                        nc.tensor.matmul(ps[:], wk0a[g][:], k0a[:],
                                         start=True, stop=False)
                        nc.tensor.matmul(ps[:], wk0b[g][:], k0b[:],
                                         start=False, stop=False)
                        for p in range(3):
                            nc.tensor.matmul(ps[:], wa[g][p][:], fA[p][:],
                                             start=False, stop=False,
                                             perf_mode=DR)
                            nc.tensor.matmul(ps[:], wb[g][p][:], fB[p][:],
                                             start=False, stop=(p == 2),
                                             perf_mode=DR)
                        if g == "r":
                            rwk = gw_pool.tile([DH, 512], F32, name="t",
                                               tag="rw")
                            nc.scalar.activation(rwk[:], ps[:], AF.Sigmoid,
                                                 bias=brt[:], scale=1.0 / SPS)
                            nc.vector.scalar_tensor_tensor(
                                rh16[b][:, cs], rwk[:], SX, hblk[:],
                                mybir.AluOpType.mult, mybir.AluOpType.mult)
                        else:
                            nc.scalar.activation(z16[b][:, cs], ps[:],
                                                 AF.Sigmoid, bias=bzt[:],
                                                 scale=1.0 / SPS)

            nc.leave_named_scope("mlp_rz", sc_rz[0], False)

            # ---------------- diffusion 2 (rh chain) ----------------
            def build_nm2():
                nms = []
                for jp in range(NJP):
                    ps = pst_pool.tile([128, 2, 512], BF16, name="t", tag="tr")
                    for h in range(2):
                        it = 2 * jp + h
                        for b in range(C):
                            nc.tensor.transpose(
                                ps[:, h, 128 * b:128 * (b + 1)],
                                rh16[b][:, 128 * it:128 * (it + 1)],
                                identb[:])
                    t = nm2_pool.tile([128, 2, 512], FP8, name="t", tag="nm2")
                    nc.vector.tensor_copy(t[:], ps[:])
                    nms.append(t)
                return nms

            with nc.named_scope("d2_hops"):
                for dirw, WP in ((0, WFP), (1, WBP)):
                    ws = load_wdir(WP)
                    cur = build_nm2()
                    for k in range(1, NHOPS + 1):
                        kidx = dirw * NHOPS + k
                        fms = hop(cur, ws, 4, k,
                                  FEAT2[(kidx - 1) // 2], (kidx - 1) % 2, 0)
                        if k < NHOPS:
                            cur = retranspose(fms, 4)

            # ---------------- MLP n + final gate ----------------
            sc_n = nc.enter_named_scope("mlp_n", False)
            for b in range(C):
                for blk in range(NBK):
                    cs = slice(512 * blk, 512 * (blk + 1))
                    k0x = k0_pool.tile([64, 512], BF16, name="t", tag="k0b")
                    nc.gpsimd.dma_start(k0x[:], XHK0[b][0:64, cs])
                    fx, fr = [], []
                    for p in range(3):
                        tX = feed_pool.tile([64, 2, 512], FP8, name="t",
                                            tag="fB")
                        nc.scalar.dma_start(
                            tX[:], FEAT1[p][b * FI:b * FI + 64, :, cs])
                        fx.append(tX)
                        tR = feed_pool.tile([128, 2, 512], FP8, name="t",
                                            tag="fA")
                        nc.gpsimd.dma_start(
                            tR[:], FEAT2[p][b * DH:b * DH + 128, :, cs])
                        fr.append(tR)
                    hblk = gw_pool.tile([DH, 512], F32, name="t", tag="h")
                    nc.sync.dma_start(hblk[:], HFM[b][:, cs])
                    ps = ps_pool.tile([128, 512], F32, name="t", tag="hop")
                    nc.tensor.matmul(ps[:], wnk0x[:], k0x[:],
                                     start=True, stop=False)
                    nc.tensor.matmul(ps[:], wnk0r[:], rh16[b][:, cs],
                                     start=False, stop=False)
                    for p in range(3):
                        nc.tensor.matmul(ps[:], wnx[p][:], fx[p][:],
                                         start=False, stop=False, perf_mode=DR)
                        nc.tensor.matmul(ps[:], wnr[p][:], fr[p][:],
                                         start=False, stop=(p == 2),
                                         perf_mode=DR)
                    nf = gw_pool.tile([DH, 512], F32, name="t", tag="nf")
                    nc.scalar.activation(nf[:], ps[:], AF.Tanh, bias=bnt[:],
                                         scale=1.0 / SPS)
                    dlt = gw_pool.tile([DH, 512], F32, name="t", tag="dw")
                    nc.vector.tensor_sub(dlt[:], nf[:], hblk[:])
                    zd = gw_pool.tile([DH, 512], F32, name="t", tag="zd")
                    nc.vector.tensor_mul(zd[:], z16[b][:, cs], dlt[:])
                    o = gw_pool.tile([DH, 512], F32, name="t", tag="o")
                    nc.vector.tensor_add(o[:], zd[:], hblk[:])
                    nc.scalar.dma_start(OUT[b][:, cs], o[:])
            nc.leave_named_scope("mlp_n", sc_n[0], False)

    nc.compile()
    return nc


_NC_CACHE = {}


def _get_nc():
    if "nc" not in _NC_CACHE:
        _NC_CACHE["nc"] = build_nc()
    return _NC_CACHE["nc"]


def _pack_gate_w(W):
    """W [128, 1344] -> (wk0 bf16 [192,128]*8192, wA fp8 [3,128,2,128]*64,
    wB fp8 [3,64,2,128]*64)."""
    W = np.asarray(W, np.float32)
    wk0 = np.ascontiguousarray((W[:, 0:FI].T * SPS)).astype(BF)
    wA = np.zeros((3, 128, 2, DH), np.float32)
    wBt = np.zeros((3, 64, 2, DH), np.float32)
    for p in range(3):
        for h in range(2):
            k = 2 * p + 1 + h
            blkc = W[:, k * FI:(k + 1) * FI]          # [128, 192]
            wA[p, :, h, :] = blkc[:, 0:128].T * SMW
            wBt[p, :, h, :] = blkc[:, 128:192].T * SMW
    return wk0, wA.astype(E4), wBt.astype(E4)


def _pack_n_w(W):
    """Wn [128, 1344] -> k0x bf16 [64,128]*8192, k0r bf16 [128,128]*512,
    wnx fp8 [3,64,2,128]*64, wnr fp8 [3,128,2,128]*64."""
    W = np.asarray(W, np.float32)
    k0x = np.ascontiguousarray(W[:, 0:64].T * SPS).astype(BF)
    k0r = np.ascontiguousarray(W[:, 64:FI].T * (SPS / SX)).astype(BF)
    wnx = np.zeros((3, 64, 2, DH), np.float32)
    wnr = np.zeros((3, 128, 2, DH), np.float32)
    for p in range(3):
        for h in range(2):
            k = 2 * p + 1 + h
            blkc = W[:, k * FI:(k + 1) * FI]
            wnx[p, :, h, :] = blkc[:, 0:64].T * SMW
            wnr[p, :, h, :] = blkc[:, 64:FI].T * SMW
    return k0x, k0r, wnx.astype(E4), wnr.astype(E4)


def _pack_wpair(W):
    """W [N,N] -> fp8 [NJP,128,2,N]: [jp,p,h,i] = 512*W[i, jp*256+h*128+p]."""
    WT = np.asarray(W, np.float32).T * SW                 # [j, i]
    N = WT.shape[0]
    return np.ascontiguousarray(
        WT.reshape(NJP, 2, 128, N).transpose(0, 2, 1, 3)).astype(E4)


def make_in_maps(x, h_prev, W_fwd, W_bwd, Wr, br, Wz, bz, Wn, bn):
    x = np.asarray(x, np.float32)
    h_prev = np.asarray(h_prev, np.float32)
    B, N, Din = x.shape
    wfp = _pack_wpair(W_fwd)
    wbp = _pack_wpair(W_bwd)
    wrk0, wrA, wrB = _pack_gate_w(Wr)
    wzk0, wzA, wzB = _pack_gate_w(Wz)
    wnk0x, wnk0r, wnx, wnr = _pack_n_w(Wn)
    identb = np.eye(128, dtype=np.float32).astype(BF)
    brc = np.ascontiguousarray(np.asarray(br, np.float32).reshape(DH, 1))
    bzc = np.ascontiguousarray(np.asarray(bz, np.float32).reshape(DH, 1))
    bnc = np.ascontiguousarray(np.asarray(bn, np.float32).reshape(DH, 1))
    ncores = B // C
    in_maps = []
    for cix in range(ncores):
        xs = x[C * cix:C * (cix + 1)]
        hs = h_prev[C * cix:C * (cix + 1)]
        xh = np.concatenate([xs, hs], axis=-1)            # [C, N, 192]
        flat = np.ascontiguousarray(xh.transpose(1, 0, 2).reshape(N, C * FI))
        xh_nm8 = np.ascontiguousarray(
            (flat * SX).reshape(NJP, 2, 128, C * FI).transpose(0, 2, 1, 3)
        ).astype(E4)
        xh_k0 = np.ascontiguousarray(xh.transpose(0, 2, 1)).astype(BF)
        h_fm = np.ascontiguousarray(hs.transpose(0, 2, 1))
        in_maps.append(dict(
            xh_nm8=xh_nm8, xh_k0=xh_k0, wfp=wfp, wbp=wbp, h_fm=h_fm,
            wrk0=wrk0, wrA=wrA, wrB=wrB, wzk0=wzk0, wzA=wzA, wzB=wzB,
            wnk0x=wnk0x, wnk0r=wnk0r, wnx=wnx, wnr=wnr,
            br_c=brc, bz_c=bzc, bn_c=bnc, identb=identb))
    return in_maps, ncores


def kernel(x, h_prev, W_fwd, W_bwd, Wr, br, Wz, bz, Wn, bn, _trace=False):
    in_maps, ncores = make_in_maps(
        x, h_prev, W_fwd, W_bwd, Wr, br, Wz, bz, Wn, bn)
    nc = _get_nc()
    res = run_bass_kernel_spmd(nc, in_maps, list(range(ncores)), trace=_trace)
    outs = [np.ascontiguousarray(res.results[c]["out_fm"].transpose(0, 2, 1))
            for c in range(ncores)]
    full = np.concatenate(outs, axis=0).astype(np.float32)
    if _trace:
        return full, res
    return full


# revision 12
# speedup vs baseline: 2.5062x; 1.0067x over previous
"""DCGRU cell on 8 Trainium2 NeuronCores — fp8 DoubleRow edition.

Sharding: data-parallel over batch (B=32 -> 4 per core), adjacency + MLP
weights replicated. No collectives; host gathers per-core outputs.

Key ideas vs the bf16 baseline:
  * Diffusion hop matmuls run in fp8e4 with MatmulPerfMode.DoubleRow: each
    instruction contracts TWO 128-row k-tiles (lhsT [128,2,M], rhs [128,2,N])
    at 0.5 cycles/output-col — 2-4x the bf16 rate.
  * Diffusion 2 only propagates the r*h feature columns (128/batch instead of
    192): the x-part hop features are identical to diffusion 1's and are
    reused for the n-gate MLP. Saves 1/3 of diffusion-2 hop FLOPs.
  * Gate logits accumulate across all 7 k-blocks in a single PSUM group per
    (batch, 512-col block) — no DRAM accumulators, no accum DMAs. Hop
    features spill to DRAM (fp8) and are gathered back per block in paired
    DoubleRow layout.
  * The k=0 MLP segments (the raw x_h / rh features, which dominate logit
    magnitude) stay bf16 with weights pre-scaled by 8192 so they share the
    PSUM accumulation group with the fp8 hop segments.

Scaling scheme (fp8e4 max normal 240):
  x_h, rh stored *16; W stored *512; hop feats stored *128.
  hop1 psum = 16*512*hop  -> copy scale 1/64  -> *128
  hopk psum = 128*512*hop -> copy scale 1/512 -> *128
  MLP hop weights *64 -> logit psum = 128*64 = 8192*logit
  k0 weights: *8192 (vs raw x_h bf16), *512 (vs rh16 bf16)
  activation scale 1/8192 recovers logits.

Per-batch feature order matches the reference concat:
  k-blocks [x_h, Wf^1, Wf^2, Wf^3, Wb^1, Wb^2, Wb^3], 192 feats each.
"""

import sys
import numpy as np
import ml_dtypes

for _p in ("/opt/trn_rl_repo",):
    if _p not in sys.path:
        sys.path.insert(0, _p)

from concourse import bacc, tile, mybir  # noqa: E402
from concourse.bass_utils import run_bass_kernel_spmd  # noqa: E402

F32 = mybir.dt.float32
BF16 = mybir.dt.bfloat16
FP8 = mybir.dt.float8e4
AF = mybir.ActivationFunctionType
DR = mybir.MatmulPerfMode.DoubleRow
E4 = ml_dtypes.float8_e4m3
BF = ml_dtypes.bfloat16

C = 4            # batches per core
FI = 192         # per-batch feature width in d1 (x 64 + h 128)
DH = 128
NCORES = 8
NHOPS = 3
NJP = 8          # node-tile pairs (N = NJP*256)
NBK = 4          # 512-col node blocks
SX = 16.0        # x_h / rh fp8 scale
SW = 512.0       # W fp8 scale
SF = 128.0       # hop-feature fp8 scale
SMW = 64.0       # MLP hop-weight fp8 scale
SPS = SF * SMW   # logit psum scale (8192)


def build_nc():
    N = NJP * 256
    nc = bacc.Bacc("TRN2", target_bir_lowering=False, debug=False,
                   num_devices=NCORES)

    def din(name, shape, dt=F32):
        return nc.dram_tensor(name, shape, dt, kind="ExternalInput").ap()

    XHNM = din("xh_nm8", [NJP, 128, 2, 768], FP8)     # 16*x_h node-major paired
    XHK0 = din("xh_k0", [C, FI, N], BF16)             # x_h feature-major raw
    WFP = din("wfp", [NJP, 128, 2, N], FP8)           # 512*W_fwd^T paired
    WBP = din("wbp", [NJP, 128, 2, N], FP8)
    HFM = din("h_fm", [C, DH, N])                     # h_prev feature-major f32
    # MLP weights (see packer)
    WK0 = {g: din(f"w{g}k0", [FI, DH], BF16) for g in ("r", "z")}
    WA = {g: din(f"w{g}A", [3, 128, 2, DH], FP8) for g in ("r", "z")}
    WB = {g: din(f"w{g}B", [3, 64, 2, DH], FP8) for g in ("r", "z")}
    WNK0X = din("wnk0x", [64, DH], BF16)
    WNK0R = din("wnk0r", [DH, DH], BF16)
    WNX = din("wnx", [3, 64, 2, DH], FP8)
    WNR = din("wnr", [3, 128, 2, DH], FP8)
    BRT = din("br_c", [DH, 1])
    BZT = din("bz_c", [DH, 1])
    BNT = din("bn_c", [DH, 1])
    IDB = din("identb", [128, 128], BF16)
    OUT = nc.dram_tensor("out_fm", [C, DH, N], F32, kind="ExternalOutput").ap()

    # k-pair-adjacent spill layouts: [pair][rows][h][N] so MLP gathers are
    # single 3-D DMAs
    FEAT1 = nc.dram_tensor("feat1", [3, 768, 2, N], FP8).ap()   # d1 feats *128
    FEAT2 = nc.dram_tensor("feat2", [3, C * DH, 2, N], FP8).ap()  # d2 feats

    with tile.TileContext(nc) as tc:
        with (
            tc.tile_pool(name="const", bufs=1) as cpool,
            tc.tile_pool(name="nmx", bufs=8) as nmx_pool,
            tc.tile_pool(name="nm1", bufs=10) as nm1_pool,
            tc.tile_pool(name="nm2", bufs=10) as nm2_pool,
            tc.tile_pool(name="wc", bufs=8) as wc_pool,
            tc.tile_pool(name="fm1", bufs=12) as fm1_pool,
            tc.tile_pool(name="fm2", bufs=8) as fm2_pool,
            tc.tile_pool(name="stg", bufs=9) as stg_pool,
            tc.tile_pool(name="feed", bufs=16) as feed_pool,
            tc.tile_pool(name="k0p", bufs=4) as k0_pool,
            tc.tile_pool(name="gw", bufs=2) as gw_pool,
            tc.tile_pool(name="gres", bufs=4) as gres_pool,
            tc.tile_pool(name="ps", bufs=6, space="PSUM") as ps_pool,
            tc.tile_pool(name="pst", bufs=2, space="PSUM") as pst_pool,
        ):
            # ---------------- constants ----------------
            identb = cpool.tile([128, 128], BF16, tag="idb")
            nc.scalar.dma_start(identb[:], IDB[:])
            brt = cpool.tile([DH, 1], F32, tag="brt")
            nc.scalar.dma_start(brt[:], BRT[:])
            bzt = cpool.tile([DH, 1], F32, tag="bzt")
            nc.scalar.dma_start(bzt[:], BZT[:])
            bnt = cpool.tile([DH, 1], F32, tag="bnt")
            nc.scalar.dma_start(bnt[:], BNT[:])
            wk0a, wk0b, wa, wb = {}, {}, {}, {}
            for g in ("r", "z"):
                wk0a[g] = cpool.tile([128, DH], BF16, name="t", tag=f"w{g}k0a")
                nc.scalar.dma_start(wk0a[g][:], WK0[g][0:128, :])
                wk0b[g] = cpool.tile([64, DH], BF16, name="t", tag=f"w{g}k0b")
                nc.scalar.dma_start(wk0b[g][:], WK0[g][128:192, :])
                wa[g] = []
                wb[g] = []
                for p in range(3):
                    t = cpool.tile([128, 2, DH], FP8, name="t", tag=f"w{g}A{p}")
                    nc.scalar.dma_start(t[:], WA[g][p])
                    wa[g].append(t)
                    t = cpool.tile([64, 2, DH], FP8, name="t", tag=f"w{g}B{p}")
                    nc.scalar.dma_start(t[:], WB[g][p])
                    wb[g].append(t)
            wnk0x = cpool.tile([64, DH], BF16, tag="wnk0x")
            nc.scalar.dma_start(wnk0x[:], WNK0X[:])
            wnk0r = cpool.tile([DH, DH], BF16, tag="wnk0r")
            nc.scalar.dma_start(wnk0r[:], WNK0R[:])
            wnx, wnr = [], []
            for p in range(3):
                t = cpool.tile([64, 2, DH], FP8, name="t", tag=f"wnx{p}")
                nc.scalar.dma_start(t[:], WNX[p])
                wnx.append(t)
                t = cpool.tile([128, 2, DH], FP8, name="t", tag=f"wnr{p}")
                nc.scalar.dma_start(t[:], WNR[p])
                wnr.append(t)

            # resident paired node-major x_h (chain start for both dirs)
            nm_xh = []
            for jp in range(NJP):
                t = nmx_pool.tile([128, 2, 768], FP8, name="t", tag="nmx")
                nc.gpsimd.dma_start(t[:], XHNM[jp])
                nm_xh.append(t)

            def load_wdir(WP):
                ws = []
                qs = [nc.sync, nc.scalar, nc.gpsimd]
                for jp in range(NJP):
                    t = wc_pool.tile([128, 2, N], FP8, name="t", tag="w")
                    qs[jp % 3].dma_start(t[:], WP[jp])
                    ws.append(t)
                return ws

            def hop(cur, ws, nch, k, FEATD, kh, rowbase):
                """One DoubleRow hop with fused per-block retransposition.
                Spills fp8 *SF feats to FEATD[:, kh, :]; for k < NHOPS also
                returns the next chain's paired fp8 nm tiles (transposes for
                jp pair (2b, 2b+1) only need block b's columns)."""
                nms = [] if k < NHOPS else None
                scale = 1.0 / 64.0 if k == 1 else 1.0 / 512.0
                for blk in range(NBK):
                    cs = slice(512 * blk, 512 * (blk + 1))
                    pss = [ps_pool.tile([128, 512], F32, name="t", tag="hop")
                           for _ in range(nch)]
                    for jp in range(NJP):
                        rhs = ws[jp][:, :, cs]
                        for c in range(nch):
                            nc.tensor.matmul(
                                pss[c][:],
                                cur[jp][:, :, 128 * c:128 * (c + 1)],
                                rhs,
                                start=(jp == 0), stop=(jp == NJP - 1),
                                perf_mode=DR)
                    fmb = None
                    if nms is not None:
                        pool = fm1_pool if nch == 6 else fm2_pool
                        tg = "fm1" if nch == 6 else "fm2"
                        fmb = [pool.tile([128, 512], BF16, name="t", tag=tg)
                               for _ in range(nch)]
                    for c in range(nch):
                        stg = stg_pool.tile([128, 512], FP8, name="t",
                                            tag="stg")
                        nc.scalar.activation(stg[:], pss[c][:], AF.Copy,
                                             scale=scale)
                        nc.sync.dma_start(
                            FEATD[rowbase + 128 * c:rowbase + 128 * (c + 1),
                                  kh, cs], stg[:])
                        if fmb is not None:
                            nc.vector.tensor_scalar_mul(fmb[c][:], pss[c][:],
                                                        scale)
                    if nms is not None:
                        for q in range(2):
                            t = (nm1_pool.tile([128, 2, 768], FP8, name="t",
                                               tag="nm1") if nch == 6 else
                                 nm2_pool.tile([128, 2, 512], FP8, name="t",
                                               tag="nm2"))
                            for h in range(2):
                                ps = pst_pool.tile([128, 128 * nch], BF16,
                                                   name="t", tag="tr")
                                for c in range(nch):
                                    nc.tensor.transpose(
                                        ps[:, 128 * c:128 * (c + 1)],
                                        fmb[c][:, 128 * (2 * q + h):
                                               128 * (2 * q + h + 1)],
                                        identb[:])
                                nc.vector.tensor_copy(t[:, h, :], ps[:])
                            nms.append(t)
                return nms

            # ---------------- diffusion 1 ----------------
            with nc.named_scope("d1_hops"):
                for dirw, WP in ((0, WFP), (1, WBP)):
                    ws = load_wdir(WP)
                    cur = nm_xh
                    for k in range(1, NHOPS + 1):
                        kidx = dirw * NHOPS + k  # 1..6
                        nxt = hop(cur, ws, 6, k,
                                  FEAT1[(kidx - 1) // 2], (kidx - 1) % 2, 0)
                        if k < NHOPS:
                            cur = nxt

            # ---------------- MLP r,z + rh ----------------
            sc_rz = nc.enter_named_scope("mlp_rz", False)
            z16 = [gres_pool.tile([DH, N], BF16, name="t", tag="z16")
                   for _ in range(C)]
            rh16 = [gres_pool.tile([DH, N], BF16, name="t", tag="rh16")
                    for _ in range(C)]
            for b in range(C):
                for blk in range(NBK):
                    cs = slice(512 * blk, 512 * (blk + 1))
                    k0a = k0_pool.tile([128, 512], BF16, name="t", tag="k0a")
                    nc.gpsimd.dma_start(k0a[:], XHK0[b][0:128, cs])
                    k0b = k0_pool.tile([64, 512], BF16, name="t", tag="k0b")
                    nc.gpsimd.dma_start(k0b[:], XHK0[b][128:192, cs])
                    fA, fB = [], []
                    for p in range(3):
                        tA = feed_pool.tile([128, 2, 512], FP8, name="t",
                                            tag="fA")
                        nc.gpsimd.dma_start(
                            tA[:], FEAT1[p][b * FI:b * FI + 128, :, cs])
                        fA.append(tA)
                        tB = feed_pool.tile([64, 2, 512], FP8, name="t",
                                            tag="fB")
                        nc.scalar.dma_start(
                            tB[:], FEAT1[p][b * FI + 128:b * FI + 192, :, cs])
                        fB.append(tB)
                    hblk = gw_pool.tile([DH, 512], F32, name="t", tag="h")
                    nc.sync.dma_start(hblk[:], HFM[b][:, cs])
                    for g in ("r", "z"):
                        ps = ps_pool.tile([128, 512], F32, name="t", tag="hop")
                        nc.tensor.matmul(ps[:], wk0a[g][:], k0a[:],
                                         start=True, stop=False)
                        nc.tensor.matmul(ps[:], wk0b[g][:], k0b[:],
                                         start=False, stop=False)
                        for p in range(3):
                            nc.tensor.matmul(ps[:], wa[g][p][:], fA[p][:],
                                             start=False, stop=False,
                                             perf_mode=DR)
                            nc.tensor.matmul(ps[:], wb[g][p][:], fB[p][:],
                                             start=False, stop=(p == 2),
                                             perf_mode=DR)
                        if g == "r":
                            rwk = gw_pool.tile([DH, 512], F32, name="t",
                                               tag="rw")
                            nc.scalar.activation(rwk[:], ps[:], AF.Sigmoid,
                                                 bias=brt[:], scale=1.0 / SPS)
                            nc.vector.scalar_tensor_tensor(
                                rh16[b][:, cs], rwk[:], SX, hblk[:],
                                mybir.AluOpType.mult, mybir.AluOpType.mult)
                        else:
                            nc.scalar.activation(z16[b][:, cs], ps[:],
                                                 AF.Sigmoid, bias=bzt[:],
                                                 scale=1.0 / SPS)

            nc.leave_named_scope("mlp_rz", sc_rz[0], False)

            # ---------------- diffusion 2 (rh chain) ----------------
            def build_nm2():
                nms = []
                for jp in range(NJP):
                    ps = pst_pool.tile([128, 2, 512], BF16, name="t", tag="tr")
                    for h in range(2):
                        it = 2 * jp + h
                        for b in range(C):
                            nc.tensor.transpose(
                                ps[:, h, 128 * b:128 * (b + 1)],
                                rh16[b][:, 128 * it:128 * (it + 1)],
                                identb[:])
                    t = nm2_pool.tile([128, 2, 512], FP8, name="t", tag="nm2")
                    nc.vector.tensor_copy(t[:], ps[:])
                    nms.append(t)
                return nms

            with nc.named_scope("d2_hops"):
                for dirw, WP in ((0, WFP), (1, WBP)):
                    ws = load_wdir(WP)
                    cur = build_nm2()
                    for k in range(1, NHOPS + 1):
                        kidx = dirw * NHOPS + k
                        nxt = hop(cur, ws, 4, k,
                                  FEAT2[(kidx - 1) // 2], (kidx - 1) % 2, 0)
                        if k < NHOPS:
                            cur = nxt

            # ---------------- MLP n + final gate ----------------
            sc_n = nc.enter_named_scope("mlp_n", False)
            for b in range(C):
                for blk in range(NBK):
                    cs = slice(512 * blk, 512 * (blk + 1))
                    k0x = k0_pool.tile([64, 512], BF16, name="t", tag="k0b")
                    nc.gpsimd.dma_start(k0x[:], XHK0[b][0:64, cs])
                    fx, fr = [], []
                    for p in range(3):
                        tX = feed_pool.tile([64, 2, 512], FP8, name="t",
                                            tag="fB")
                        nc.scalar.dma_start(
                            tX[:], FEAT1[p][b * FI:b * FI + 64, :, cs])
                        fx.append(tX)
                        tR = feed_pool.tile([128, 2, 512], FP8, name="t",
                                            tag="fA")
                        nc.gpsimd.dma_start(
                            tR[:], FEAT2[p][b * DH:b * DH + 128, :, cs])
                        fr.append(tR)
                    hblk = gw_pool.tile([DH, 512], F32, name="t", tag="h")
                    nc.sync.dma_start(hblk[:], HFM[b][:, cs])
                    ps = ps_pool.tile([128, 512], F32, name="t", tag="hop")
                    nc.tensor.matmul(ps[:], wnk0x[:], k0x[:],
                                     start=True, stop=False)
                    nc.tensor.matmul(ps[:], wnk0r[:], rh16[b][:, cs],
                                     start=False, stop=False)
                    for p in range(3):
                        nc.tensor.matmul(ps[:], wnx[p][:], fx[p][:],
                                         start=False, stop=False, perf_mode=DR)
                        nc.tensor.matmul(ps[:], wnr[p][:], fr[p][:],
                                         start=False, stop=(p == 2),
                                         perf_mode=DR)
                    nf = gw_pool.tile([DH, 512], F32, name="t", tag="nf")
                    nc.scalar.activation(nf[:], ps[:], AF.Tanh, bias=bnt[:],
                                         scale=1.0 / SPS)
                    dlt = gw_pool.tile([DH, 512], F32, name="t", tag="dw")
                    nc.vector.tensor_sub(dlt[:], nf[:], hblk[:])
                    zd = gw_pool.tile([DH, 512], F32, name="t", tag="zd")
                    nc.vector.tensor_mul(zd[:], z16[b][:, cs], dlt[:])
                    o = gw_pool.tile([DH, 512], F32, name="t", tag="o")
                    nc.vector.tensor_add(o[:], zd[:], hblk[:])
                    nc.scalar.dma_start(OUT[b][:, cs], o[:])
            nc.leave_named_scope("mlp_n", sc_n[0], False)

    nc.compile()
    return nc


_NC_CACHE = {}


def _get_nc():
    if "nc" not in _NC_CACHE:
        _NC_CACHE["nc"] = build_nc()
    return _NC_CACHE["nc"]


def _pack_gate_w(W):
    """W [128, 1344] -> (wk0 bf16 [192,128]*8192, wA fp8 [3,128,2,128]*64,
    wB fp8 [3,64,2,128]*64)."""
    W = np.asarray(W, np.float32)
    wk0 = np.ascontiguousarray((W[:, 0:FI].T * SPS)).astype(BF)
    wA = np.zeros((3, 128, 2, DH), np.float32)
    wBt = np.zeros((3, 64, 2, DH), np.float32)
    for p in range(3):
        for h in range(2):
            k = 2 * p + 1 + h
            blkc = W[:, k * FI:(k + 1) * FI]          # [128, 192]
            wA[p, :, h, :] = blkc[:, 0:128].T * SMW
            wBt[p, :, h, :] = blkc[:, 128:192].T * SMW
    return wk0, wA.astype(E4), wBt.astype(E4)


def _pack_n_w(W):
    """Wn [128, 1344] -> k0x bf16 [64,128]*8192, k0r bf16 [128,128]*512,
    wnx fp8 [3,64,2,128]*64, wnr fp8 [3,128,2,128]*64."""
    W = np.asarray(W, np.float32)
    k0x = np.ascontiguousarray(W[:, 0:64].T * SPS).astype(BF)
    k0r = np.ascontiguousarray(W[:, 64:FI].T * (SPS / SX)).astype(BF)
    wnx = np.zeros((3, 64, 2, DH), np.float32)
    wnr = np.zeros((3, 128, 2, DH), np.float32)
    for p in range(3):
        for h in range(2):
            k = 2 * p + 1 + h
            blkc = W[:, k * FI:(k + 1) * FI]
            wnx[p, :, h, :] = blkc[:, 0:64].T * SMW
            wnr[p, :, h, :] = blkc[:, 64:FI].T * SMW
    return k0x, k0r, wnx.astype(E4), wnr.astype(E4)


def _pack_wpair(W):
    """W [N,N] -> fp8 [NJP,128,2,N]: [jp,p,h,i] = 512*W[i, jp*256+h*128+p]."""
    WT = np.asarray(W, np.float32).T * SW                 # [j, i]
    N = WT.shape[0]
    return np.ascontiguousarray(
        WT.reshape(NJP, 2, 128, N).transpose(0, 2, 1, 3)).astype(E4)


def make_in_maps(x, h_prev, W_fwd, W_bwd, Wr, br, Wz, bz, Wn, bn):
    x = np.asarray(x, np.float32)
    h_prev = np.asarray(h_prev, np.float32)
    B, N, Din = x.shape
    wfp = _pack_wpair(W_fwd)
    wbp = _pack_wpair(W_bwd)
    wrk0, wrA, wrB = _pack_gate_w(Wr)
    wzk0, wzA, wzB = _pack_gate_w(Wz)
    wnk0x, wnk0r, wnx, wnr = _pack_n_w(Wn)
    identb = np.eye(128, dtype=np.float32).astype(BF)
    brc = np.ascontiguousarray(np.asarray(br, np.float32).reshape(DH, 1))
    bzc = np.ascontiguousarray(np.asarray(bz, np.float32).reshape(DH, 1))
    bnc = np.ascontiguousarray(np.asarray(bn, np.float32).reshape(DH, 1))
    ncores = B // C
    in_maps = []
    for cix in range(ncores):
        xs = x[C * cix:C * (cix + 1)]
        hs = h_prev[C * cix:C * (cix + 1)]
        xh = np.concatenate([xs, hs], axis=-1)            # [C, N, 192]
        flat = np.ascontiguousarray(xh.transpose(1, 0, 2).reshape(N, C * FI))
        xh_nm8 = np.ascontiguousarray(
            (flat * SX).reshape(NJP, 2, 128, C * FI).transpose(0, 2, 1, 3)
        ).astype(E4)
        xh_k0 = np.ascontiguousarray(xh.transpose(0, 2, 1)).astype(BF)
        h_fm = np.ascontiguousarray(hs.transpose(0, 2, 1))
        in_maps.append(dict(
            xh_nm8=xh_nm8, xh_k0=xh_k0, wfp=wfp, wbp=wbp, h_fm=h_fm,
            wrk0=wrk0, wrA=wrA, wrB=wrB, wzk0=wzk0, wzA=wzA, wzB=wzB,
            wnk0x=wnk0x, wnk0r=wnk0r, wnx=wnx, wnr=wnr,
            br_c=brc, bz_c=bzc, bn_c=bnc, identb=identb))
    return in_maps, ncores


def kernel(x, h_prev, W_fwd, W_bwd, Wr, br, Wz, bz, Wn, bn, _trace=False):
    in_maps, ncores = make_in_maps(
        x, h_prev, W_fwd, W_bwd, Wr, br, Wz, bz, Wn, bn)
    nc = _get_nc()
    res = run_bass_kernel_spmd(nc, in_maps, list(range(ncores)), trace=_trace)
    outs = [np.ascontiguousarray(res.results[c]["out_fm"].transpose(0, 2, 1))
            for c in range(ncores)]
    full = np.concatenate(outs, axis=0).astype(np.float32)
    if _trace:
        return full, res
    return full


# revision 14
# speedup vs baseline: 2.7050x; 1.0793x over previous
"""DCGRU cell on 8 Trainium2 NeuronCores — fp8 DoubleRow edition.

Sharding: data-parallel over batch (B=32 -> 4 per core), adjacency + MLP
weights replicated. No collectives; host gathers per-core outputs.

Key ideas vs the bf16 baseline:
  * Diffusion hop matmuls run in fp8e4 with MatmulPerfMode.DoubleRow: each
    instruction contracts TWO 128-row k-tiles (lhsT [128,2,M], rhs [128,2,N])
    at 0.5 cycles/output-col — 2-4x the bf16 rate.
  * Diffusion 2 only propagates the r*h feature columns (128/batch instead of
    192): the x-part hop features are identical to diffusion 1's and are
    reused for the n-gate MLP. Saves 1/3 of diffusion-2 hop FLOPs.
  * Gate logits accumulate across all 7 k-blocks in a single PSUM group per
    (batch, 512-col block) — no DRAM accumulators, no accum DMAs. Hop
    features spill to DRAM (fp8) and are gathered back per block in paired
    DoubleRow layout.
  * The k=0 MLP segments (the raw x_h / rh features, which dominate logit
    magnitude) stay bf16 with weights pre-scaled by 8192 so they share the
    PSUM accumulation group with the fp8 hop segments.

Scaling scheme (fp8e4 max normal 240):
  x_h, rh stored *16; W stored *512; hop feats stored *128.
  hop1 psum = 16*512*hop  -> copy scale 1/64  -> *128
  hopk psum = 128*512*hop -> copy scale 1/512 -> *128
  MLP hop weights *64 -> logit psum = 128*64 = 8192*logit
  k0 weights: *8192 (vs raw x_h bf16), *512 (vs rh16 bf16)
  activation scale 1/8192 recovers logits.

Per-batch feature order matches the reference concat:
  k-blocks [x_h, Wf^1, Wf^2, Wf^3, Wb^1, Wb^2, Wb^3], 192 feats each.
"""

import sys
import numpy as np
import ml_dtypes

for _p in ("/opt/trn_rl_repo",):
    if _p not in sys.path:
        sys.path.insert(0, _p)

from concourse import bacc, tile, mybir  # noqa: E402
from concourse.bass_utils import run_bass_kernel_spmd  # noqa: E402

F32 = mybir.dt.float32
BF16 = mybir.dt.bfloat16
FP8 = mybir.dt.float8e4
AF = mybir.ActivationFunctionType
DR = mybir.MatmulPerfMode.DoubleRow
E4 = ml_dtypes.float8_e4m3
BF = ml_dtypes.bfloat16

C = 4            # batches per core
FI = 192         # per-batch feature width in d1 (x 64 + h 128)
DH = 128
NCORES = 8
NHOPS = 3
NJP = 8          # node-tile pairs (N = NJP*256)
NBK = 4          # 512-col node blocks
SX = 16.0        # x_h / rh fp8 scale
SW = 512.0       # W fp8 scale
SF = 128.0       # hop-feature fp8 scale
SMW = 64.0       # MLP hop-weight fp8 scale
SPS = SF * SMW   # logit psum scale (8192)


def build_nc():
    N = NJP * 256
    nc = bacc.Bacc("TRN2", target_bir_lowering=False, debug=False,
                   num_devices=NCORES)

    def din(name, shape, dt=F32):
        return nc.dram_tensor(name, shape, dt, kind="ExternalInput").ap()

    XHNM = din("xh_nm8", [NJP, 128, 2, 768], FP8)     # 16*x_h node-major paired
    XHK0 = din("xh_k0", [C, FI, N], BF16)             # x_h feature-major raw
    WFP = din("wfp", [NJP, 128, 2, N], FP8)           # 512*W_fwd^T paired
    WBP = din("wbp", [NJP, 128, 2, N], FP8)
    HFM = din("h_fm", [C, DH, N])                     # h_prev feature-major f32
    # MLP weights (see packer)
    WK0 = {g: din(f"w{g}k0", [FI, DH], BF16) for g in ("r", "z")}
    WA = {g: din(f"w{g}A", [3, 128, 2, DH], FP8) for g in ("r", "z")}
    WB = {g: din(f"w{g}B", [3, 64, 2, DH], FP8) for g in ("r", "z")}
    WNK0X = din("wnk0x", [64, DH], BF16)
    WNK0R = din("wnk0r", [DH, DH], BF16)
    WNX = din("wnx", [3, 64, 2, DH], FP8)
    WNR = din("wnr", [3, 128, 2, DH], FP8)
    BRT = din("br_c", [DH, 1])
    BZT = din("bz_c", [DH, 1])
    BNT = din("bn_c", [DH, 1])
    IDB = din("identb", [128, 128], BF16)
    OUT = nc.dram_tensor("out_fm", [C, DH, N], F32, kind="ExternalOutput").ap()

    # k-pair-adjacent spill layouts: [pair][rows][h][N] so MLP gathers are
    # single 3-D DMAs
    FEAT1 = nc.dram_tensor("feat1", [3, 768, 2, N], FP8).ap()   # d1 feats *128
    FEAT2 = nc.dram_tensor("feat2", [3, C * DH, 2, N], FP8).ap()  # d2 feats

    with tile.TileContext(nc) as tc:
        with (
            tc.tile_pool(name="const", bufs=1) as cpool,
            tc.tile_pool(name="nmx", bufs=8) as nmx_pool,
            tc.tile_pool(name="nm1", bufs=10) as nm1_pool,
            tc.tile_pool(name="nm2", bufs=10) as nm2_pool,
            tc.tile_pool(name="wc", bufs=8) as wc_pool,
            tc.tile_pool(name="fm1", bufs=12) as fm1_pool,
            tc.tile_pool(name="fm2", bufs=8) as fm2_pool,
            tc.tile_pool(name="stg", bufs=9) as stg_pool,
            tc.tile_pool(name="feed", bufs=16) as feed_pool,
            tc.tile_pool(name="k0p", bufs=4) as k0_pool,
            tc.tile_pool(name="gw", bufs=2) as gw_pool,
            tc.tile_pool(name="gres", bufs=4) as gres_pool,
            tc.tile_pool(name="ps", bufs=6, space="PSUM") as ps_pool,
            tc.tile_pool(name="pst", bufs=2, space="PSUM") as pst_pool,
        ):
            # ---------------- constants ----------------
            identb = cpool.tile([128, 128], BF16, tag="idb")
            nc.scalar.dma_start(identb[:], IDB[:])
            brt = cpool.tile([DH, 1], F32, tag="brt")
            nc.scalar.dma_start(brt[:], BRT[:])
            bzt = cpool.tile([DH, 1], F32, tag="bzt")
            nc.scalar.dma_start(bzt[:], BZT[:])
            bnt = cpool.tile([DH, 1], F32, tag="bnt")
            nc.scalar.dma_start(bnt[:], BNT[:])
            wk0a, wk0b, wa, wb = {}, {}, {}, {}
            for g in ("r", "z"):
                wk0a[g] = cpool.tile([128, DH], BF16, name="t", tag=f"w{g}k0a")
                nc.scalar.dma_start(wk0a[g][:], WK0[g][0:128, :])
                wk0b[g] = cpool.tile([64, DH], BF16, name="t", tag=f"w{g}k0b")
                nc.scalar.dma_start(wk0b[g][:], WK0[g][128:192, :])
                wa[g] = []
                wb[g] = []
                for p in range(3):
                    t = cpool.tile([128, 2, DH], FP8, name="t", tag=f"w{g}A{p}")
                    nc.scalar.dma_start(t[:], WA[g][p])
                    wa[g].append(t)
                    t = cpool.tile([64, 2, DH], FP8, name="t", tag=f"w{g}B{p}")
                    nc.scalar.dma_start(t[:], WB[g][p])
                    wb[g].append(t)
            wnk0x = cpool.tile([64, DH], BF16, tag="wnk0x")
            nc.scalar.dma_start(wnk0x[:], WNK0X[:])
            wnk0r = cpool.tile([DH, DH], BF16, tag="wnk0r")
            nc.scalar.dma_start(wnk0r[:], WNK0R[:])
            wnx, wnr = [], []
            for p in range(3):
                t = cpool.tile([64, 2, DH], FP8, name="t", tag=f"wnx{p}")
                nc.scalar.dma_start(t[:], WNX[p])
                wnx.append(t)
                t = cpool.tile([128, 2, DH], FP8, name="t", tag=f"wnr{p}")
                nc.scalar.dma_start(t[:], WNR[p])
                wnr.append(t)

            # resident paired node-major x_h (chain start for both dirs)
            nm_xh = []
            _qs = [nc.sync, nc.scalar, nc.gpsimd]
            for jp in range(NJP):
                t = nmx_pool.tile([128, 2, 768], FP8, name="t", tag="nmx")
                _qs[jp % 3].dma_start(t[:], XHNM[jp])
                nm_xh.append(t)

            def load_wdir(WP):
                """Block-major striped load: all jp's block-0 slices land
                first so the next hop's first 512-col block starts after
                ~1MB instead of the full 4.2MB."""
                ws = []
                qs = [nc.sync, nc.scalar, nc.gpsimd]
                for jp in range(NJP):
                    t = wc_pool.tile([128, 2, N], FP8, name="t", tag="w")
                    ws.append(t)
                for blk in range(NBK):
                    cs = slice(512 * blk, 512 * (blk + 1))
                    for jp in range(NJP):
                        qs[jp % 3].dma_start(ws[jp][:, :, cs],
                                             WP[jp][:, :, cs])
                return ws

            def hop(cur, ws, nch, k, FEATD, kh, rowbase):
                """One DoubleRow hop with fused per-block retransposition.
                Spills fp8 *SF feats to FEATD[:, kh, :]; for k < NHOPS also
                returns the next chain's paired fp8 nm tiles (transposes for
                jp pair (2b, 2b+1) only need block b's columns)."""
                nms = [] if k < NHOPS else None
                scale = 1.0 / 64.0 if k == 1 else 1.0 / 512.0
                for blk in range(NBK):
                    cs = slice(512 * blk, 512 * (blk + 1))
                    pss = [ps_pool.tile([128, 512], F32, name="t", tag="hop")
                           for _ in range(nch)]
                    for jp in range(NJP):
                        rhs = ws[jp][:, :, cs]
                        for c in range(nch):
                            nc.tensor.matmul(
                                pss[c][:],
                                cur[jp][:, :, 128 * c:128 * (c + 1)],
                                rhs,
                                start=(jp == 0), stop=(jp == NJP - 1),
                                perf_mode=DR)
                    fmb = None
                    if nms is not None:
                        pool = fm1_pool if nch == 6 else fm2_pool
                        tg = "fm1" if nch == 6 else "fm2"
                        fmb = [pool.tile([128, 512], BF16, name="t", tag=tg)
                               for _ in range(nch)]
                    for c in range(nch):
                        stg = stg_pool.tile([128, 512], FP8, name="t",
                                            tag="stg")
                        nc.scalar.activation(stg[:], pss[c][:], AF.Copy,
                                             scale=scale)
                        nc.sync.dma_start(
                            FEATD[rowbase + 128 * c:rowbase + 128 * (c + 1),
                                  kh, cs], stg[:])
                        if fmb is not None:
                            nc.vector.tensor_scalar_mul(fmb[c][:], pss[c][:],
                                                        scale)
                    if nms is not None:
                        for q in range(2):
                            t = (nm1_pool.tile([128, 2, 768], FP8, name="t",
                                               tag="nm1") if nch == 6 else
                                 nm2_pool.tile([128, 2, 512], FP8, name="t",
                                               tag="nm2"))
                            for h in range(2):
                                ps = pst_pool.tile([128, 128 * nch], BF16,
                                                   name="t", tag="tr")
                                for c in range(nch):
                                    nc.tensor.transpose(
                                        ps[:, 128 * c:128 * (c + 1)],
                                        fmb[c][:, 128 * (2 * q + h):
                                               128 * (2 * q + h + 1)],
                                        identb[:])
                                nc.vector.tensor_copy(t[:, h, :], ps[:])
                            nms.append(t)
                return nms

            # ---------------- diffusion 1 ----------------
            with nc.named_scope("d1_hops"):
                for dirw, WP in ((0, WFP), (1, WBP)):
                    ws = load_wdir(WP)
                    cur = nm_xh
                    for k in range(1, NHOPS + 1):
                        kidx = dirw * NHOPS + k  # 1..6
                        nxt = hop(cur, ws, 6, k,
                                  FEAT1[(kidx - 1) // 2], (kidx - 1) % 2, 0)
                        if k < NHOPS:
                            cur = nxt

            # ---------------- MLP r,z + rh ----------------
            sc_rz = nc.enter_named_scope("mlp_rz", False)
            z16 = [gres_pool.tile([DH, N], BF16, name="t", tag="z16")
                   for _ in range(C)]
            rh16 = [gres_pool.tile([DH, N], BF16, name="t", tag="rh16")
                    for _ in range(C)]
            for b in range(C):
                for blk in range(NBK):
                    cs = slice(512 * blk, 512 * (blk + 1))
                    k0a = k0_pool.tile([128, 512], BF16, name="t", tag="k0a")
                    nc.gpsimd.dma_start(k0a[:], XHK0[b][0:128, cs])
                    k0b = k0_pool.tile([64, 512], BF16, name="t", tag="k0b")
                    nc.gpsimd.dma_start(k0b[:], XHK0[b][128:192, cs])
                    fA, fB = [], []
                    for p in range(3):
                        tA = feed_pool.tile([128, 2, 512], FP8, name="t",
                                            tag="fA")
                        nc.gpsimd.dma_start(
                            tA[:], FEAT1[p][b * FI:b * FI + 128, :, cs])
                        fA.append(tA)
                        tB = feed_pool.tile([64, 2, 512], FP8, name="t",
                                            tag="fB")
                        nc.scalar.dma_start(
                            tB[:], FEAT1[p][b * FI + 128:b * FI + 192, :, cs])
                        fB.append(tB)
                    hblk = gw_pool.tile([DH, 512], F32, name="t", tag="h")
                    nc.sync.dma_start(hblk[:], HFM[b][:, cs])
                    for g in ("r", "z"):
                        ps = ps_pool.tile([128, 512], F32, name="t", tag="hop")
                        nc.tensor.matmul(ps[:], wk0a[g][:], k0a[:],
                                         start=True, stop=False)
                        nc.tensor.matmul(ps[:], wk0b[g][:], k0b[:],
                                         start=False, stop=False)
                        for p in range(3):
                            nc.tensor.matmul(ps[:], wa[g][p][:], fA[p][:],
                                             start=False, stop=False,
                                             perf_mode=DR)
                            nc.tensor.matmul(ps[:], wb[g][p][:], fB[p][:],
                                             start=False, stop=(p == 2),
                                             perf_mode=DR)
                        if g == "r":
                            rwk = gw_pool.tile([DH, 512], F32, name="t",
                                               tag="rw")
                            nc.scalar.activation(rwk[:], ps[:], AF.Sigmoid,
                                                 bias=brt[:], scale=1.0 / SPS)
                            nc.vector.scalar_tensor_tensor(
                                rh16[b][:, cs], rwk[:], SX, hblk[:],
                                mybir.AluOpType.mult, mybir.AluOpType.mult)
                        else:
                            nc.scalar.activation(z16[b][:, cs], ps[:],
                                                 AF.Sigmoid, bias=bzt[:],
                                                 scale=1.0 / SPS)

            nc.leave_named_scope("mlp_rz", sc_rz[0], False)

            # ---------------- diffusion 2 (rh chain) ----------------
            def build_nm2():
                nms = []
                for jp in range(NJP):
                    ps = pst_pool.tile([128, 2, 512], BF16, name="t", tag="tr")
                    for h in range(2):
                        it = 2 * jp + h
                        for b in range(C):
                            nc.tensor.transpose(
                                ps[:, h, 128 * b:128 * (b + 1)],
                                rh16[b][:, 128 * it:128 * (it + 1)],
                                identb[:])
                    t = nm2_pool.tile([128, 2, 512], FP8, name="t", tag="nm2")
                    nc.vector.tensor_copy(t[:], ps[:])
                    nms.append(t)
                return nms

            with nc.named_scope("d2_hops"):
                for dirw, WP in ((0, WFP), (1, WBP)):
                    ws = load_wdir(WP)
                    cur = build_nm2()
                    for k in range(1, NHOPS + 1):
                        kidx = dirw * NHOPS + k
                        nxt = hop(cur, ws, 4, k,
                                  FEAT2[(kidx - 1) // 2], (kidx - 1) % 2, 0)
                        if k < NHOPS:
                            cur = nxt

            # ---------------- MLP n + final gate ----------------
            sc_n = nc.enter_named_scope("mlp_n", False)
            for b in range(C):
                for blk in range(NBK):
                    cs = slice(512 * blk, 512 * (blk + 1))
                    k0x = k0_pool.tile([64, 512], BF16, name="t", tag="k0b")
                    nc.gpsimd.dma_start(k0x[:], XHK0[b][0:64, cs])
                    fx, fr = [], []
                    for p in range(3):
                        tX = feed_pool.tile([64, 2, 512], FP8, name="t",
                                            tag="fB")
                        nc.scalar.dma_start(
                            tX[:], FEAT1[p][b * FI:b * FI + 64, :, cs])
                        fx.append(tX)
                        tR = feed_pool.tile([128, 2, 512], FP8, name="t",
                                            tag="fA")
                        nc.gpsimd.dma_start(
                            tR[:], FEAT2[p][b * DH:b * DH + 128, :, cs])
                        fr.append(tR)
                    hblk = gw_pool.tile([DH, 512], F32, name="t", tag="h")
                    nc.sync.dma_start(hblk[:], HFM[b][:, cs])
                    ps = ps_pool.tile([128, 512], F32, name="t", tag="hop")
                    nc.tensor.matmul(ps[:], wnk0x[:], k0x[:],
                                     start=True, stop=False)
                    nc.tensor.matmul(ps[:], wnk0r[:], rh16[b][:, cs],
                                     start=False, stop=False)
                    for p in range(3):
                        nc.tensor.matmul(ps[:], wnx[p][:], fx[p][:],
                                         start=False, stop=False, perf_mode=DR)
                        nc.tensor.matmul(ps[:], wnr[p][:], fr[p][:],
                                         start=False, stop=(p == 2),
                                         perf_mode=DR)
                    nf = gw_pool.tile([DH, 512], F32, name="t", tag="nf")
                    nc.scalar.activation(nf[:], ps[:], AF.Tanh, bias=bnt[:],
                                         scale=1.0 / SPS)
                    dlt = gw_pool.tile([DH, 512], F32, name="t", tag="dw")
                    nc.vector.tensor_sub(dlt[:], nf[:], hblk[:])
                    zd = gw_pool.tile([DH, 512], F32, name="t", tag="zd")
                    nc.vector.tensor_mul(zd[:], z16[b][:, cs], dlt[:])
                    o = gw_pool.tile([DH, 512], F32, name="t", tag="o")
                    nc.vector.tensor_add(o[:], zd[:], hblk[:])
                    nc.scalar.dma_start(OUT[b][:, cs], o[:])
            nc.leave_named_scope("mlp_n", sc_n[0], False)

    nc.compile()
    return nc


_NC_CACHE = {}


def _get_nc():
    if "nc" not in _NC_CACHE:
        _NC_CACHE["nc"] = build_nc()
    return _NC_CACHE["nc"]


def _pack_gate_w(W):
    """W [128, 1344] -> (wk0 bf16 [192,128]*8192, wA fp8 [3,128,2,128]*64,
    wB fp8 [3,64,2,128]*64)."""
    W = np.asarray(W, np.float32)
    wk0 = np.ascontiguousarray((W[:, 0:FI].T * SPS)).astype(BF)
    wA = np.zeros((3, 128, 2, DH), np.float32)
    wBt = np.zeros((3, 64, 2, DH), np.float32)
    for p in range(3):
        for h in range(2):
            k = 2 * p + 1 + h
            blkc = W[:, k * FI:(k + 1) * FI]          # [128, 192]
            wA[p, :, h, :] = blkc[:, 0:128].T * SMW
            wBt[p, :, h, :] = blkc[:, 128:192].T * SMW
    return wk0, wA.astype(E4), wBt.astype(E4)


def _pack_n_w(W):
    """Wn [128, 1344] -> k0x bf16 [64,128]*8192, k0r bf16 [128,128]*512,
    wnx fp8 [3,64,2,128]*64, wnr fp8 [3,128,2,128]*64."""
    W = np.asarray(W, np.float32)
    k0x = np.ascontiguousarray(W[:, 0:64].T * SPS).astype(BF)
    k0r = np.ascontiguousarray(W[:, 64:FI].T * (SPS / SX)).astype(BF)
    wnx = np.zeros((3, 64, 2, DH), np.float32)
    wnr = np.zeros((3, 128, 2, DH), np.float32)
    for p in range(3):
        for h in range(2):
            k = 2 * p + 1 + h
            blkc = W[:, k * FI:(k + 1) * FI]
            wnx[p, :, h, :] = blkc[:, 0:64].T * SMW
            wnr[p, :, h, :] = blkc[:, 64:FI].T * SMW
    return k0x, k0r, wnx.astype(E4), wnr.astype(E4)


def _pack_wpair(W):
    """W [N,N] -> fp8 [NJP,128,2,N]: [jp,p,h,i] = 512*W[i, jp*256+h*128+p]."""
    WT = np.asarray(W, np.float32).T * SW                 # [j, i]
    N = WT.shape[0]
    return np.ascontiguousarray(
        WT.reshape(NJP, 2, 128, N).transpose(0, 2, 1, 3)).astype(E4)


def make_in_maps(x, h_prev, W_fwd, W_bwd, Wr, br, Wz, bz, Wn, bn):
    x = np.asarray(x, np.float32)
    h_prev = np.asarray(h_prev, np.float32)
    B, N, Din = x.shape
    wfp = _pack_wpair(W_fwd)
    wbp = _pack_wpair(W_bwd)
    wrk0, wrA, wrB = _pack_gate_w(Wr)
    wzk0, wzA, wzB = _pack_gate_w(Wz)
    wnk0x, wnk0r, wnx, wnr = _pack_n_w(Wn)
    identb = np.eye(128, dtype=np.float32).astype(BF)
    brc = np.ascontiguousarray(np.asarray(br, np.float32).reshape(DH, 1))
    bzc = np.ascontiguousarray(np.asarray(bz, np.float32).reshape(DH, 1))
    bnc = np.ascontiguousarray(np.asarray(bn, np.float32).reshape(DH, 1))
    ncores = B // C
    in_maps = []
    for cix in range(ncores):
        xs = x[C * cix:C * (cix + 1)]
        hs = h_prev[C * cix:C * (cix + 1)]
        xh = np.concatenate([xs, hs], axis=-1)            # [C, N, 192]
        flat = np.ascontiguousarray(xh.transpose(1, 0, 2).reshape(N, C * FI))
        xh_nm8 = np.ascontiguousarray(
            (flat * SX).reshape(NJP, 2, 128, C * FI).transpose(0, 2, 1, 3)
        ).astype(E4)
        xh_k0 = np.ascontiguousarray(xh.transpose(0, 2, 1)).astype(BF)
        h_fm = np.ascontiguousarray(hs.transpose(0, 2, 1))
        in_maps.append(dict(
            xh_nm8=xh_nm8, xh_k0=xh_k0, wfp=wfp, wbp=wbp, h_fm=h_fm,
            wrk0=wrk0, wrA=wrA, wrB=wrB, wzk0=wzk0, wzA=wzA, wzB=wzB,
            wnk0x=wnk0x, wnk0r=wnk0r, wnx=wnx, wnr=wnr,
            br_c=brc, bz_c=bzc, bn_c=bnc, identb=identb))
    return in_maps, ncores


def kernel(x, h_prev, W_fwd, W_bwd, Wr, br, Wz, bz, Wn, bn, _trace=False):
    in_maps, ncores = make_in_maps(
        x, h_prev, W_fwd, W_bwd, Wr, br, Wz, bz, Wn, bn)
    nc = _get_nc()
    res = run_bass_kernel_spmd(nc, in_maps, list(range(ncores)), trace=_trace)
    outs = [np.ascontiguousarray(res.results[c]["out_fm"].transpose(0, 2, 1))
            for c in range(ncores)]
    full = np.concatenate(outs, axis=0).astype(np.float32)
    if _trace:
        return full, res
    return full


# revision 15
# speedup vs baseline: 2.7396x; 1.0128x over previous
"""DCGRU cell on 8 Trainium2 NeuronCores — fp8 DoubleRow edition.

Sharding: data-parallel over batch (B=32 -> 4 per core), adjacency + MLP
weights replicated. No collectives; host gathers per-core outputs.

Key ideas vs the bf16 baseline:
  * Diffusion hop matmuls run in fp8e4 with MatmulPerfMode.DoubleRow: each
    instruction contracts TWO 128-row k-tiles (lhsT [128,2,M], rhs [128,2,N])
    at 0.5 cycles/output-col — 2-4x the bf16 rate.
  * Diffusion 2 only propagates the r*h feature columns (128/batch instead of
    192): the x-part hop features are identical to diffusion 1's and are
    reused for the n-gate MLP. Saves 1/3 of diffusion-2 hop FLOPs.
  * Gate logits accumulate across all 7 k-blocks in a single PSUM group per
    (batch, 512-col block) — no DRAM accumulators, no accum DMAs. Hop
    features spill to DRAM (fp8) and are gathered back per block in paired
    DoubleRow layout.
  * The k=0 MLP segments (the raw x_h / rh features, which dominate logit
    magnitude) stay bf16 with weights pre-scaled by 8192 so they share the
    PSUM accumulation group with the fp8 hop segments.

Scaling scheme (fp8e4 max normal 240):
  x_h, rh stored *16; W stored *512; hop feats stored *128.
  hop1 psum = 16*512*hop  -> copy scale 1/64  -> *128
  hopk psum = 128*512*hop -> copy scale 1/512 -> *128
  MLP hop weights *64 -> logit psum = 128*64 = 8192*logit
  k0 weights: *8192 (vs raw x_h bf16), *512 (vs rh16 bf16)
  activation scale 1/8192 recovers logits.

Per-batch feature order matches the reference concat:
  k-blocks [x_h, Wf^1, Wf^2, Wf^3, Wb^1, Wb^2, Wb^3], 192 feats each.
"""

import sys
import numpy as np
import ml_dtypes

for _p in ("/opt/trn_rl_repo",):
    if _p not in sys.path:
        sys.path.insert(0, _p)

from concourse import bacc, tile, mybir  # noqa: E402
from concourse.bass_utils import run_bass_kernel_spmd  # noqa: E402

F32 = mybir.dt.float32
BF16 = mybir.dt.bfloat16
FP8 = mybir.dt.float8e4
AF = mybir.ActivationFunctionType
DR = mybir.MatmulPerfMode.DoubleRow
E4 = ml_dtypes.float8_e4m3
BF = ml_dtypes.bfloat16

C = 4            # batches per core
FI = 192         # per-batch feature width in d1 (x 64 + h 128)
DH = 128
NCORES = 8
NHOPS = 3
NJP = 8          # node-tile pairs (N = NJP*256)
NBK = 4          # 512-col node blocks
SX = 16.0        # x_h / rh fp8 scale
SW = 512.0       # W fp8 scale
SF = 128.0       # hop-feature fp8 scale
SMW = 64.0       # MLP hop-weight fp8 scale
SPS = SF * SMW   # logit psum scale (8192)


def build_nc():
    N = NJP * 256
    nc = bacc.Bacc("TRN2", target_bir_lowering=False, debug=False,
                   num_devices=NCORES)

    def din(name, shape, dt=F32):
        return nc.dram_tensor(name, shape, dt, kind="ExternalInput").ap()

    XHNM = din("xh_nm8", [NJP, 128, 2, 768], FP8)     # 16*x_h node-major paired
    XHK0 = din("xh_k0", [C, FI, N], BF16)             # x_h feature-major raw
    WFP = din("wfp", [NJP, 128, 2, N], FP8)           # 512*W_fwd^T paired
    WBP = din("wbp", [NJP, 128, 2, N], FP8)
    HFM = din("h_fm", [C, DH, N])                     # h_prev feature-major f32
    # MLP weights (see packer)
    WK0 = {g: din(f"w{g}k0", [FI, DH], BF16) for g in ("r", "z")}
    WA = {g: din(f"w{g}A", [3, 128, 2, DH], FP8) for g in ("r", "z")}
    WB = {g: din(f"w{g}B", [3, 64, 2, DH], FP8) for g in ("r", "z")}
    WNK0X = din("wnk0x", [64, DH], BF16)
    WNK0R = din("wnk0r", [DH, DH], BF16)
    WNX = din("wnx", [3, 64, 2, DH], FP8)
    WNR = din("wnr", [3, 128, 2, DH], FP8)
    BRT = din("br_c", [DH, 1])
    BZT = din("bz_c", [DH, 1])
    BNT = din("bn_c", [DH, 1])
    IDB = din("identb", [128, 128], BF16)
    OUT = nc.dram_tensor("out_fm", [C, DH, N], F32, kind="ExternalOutput").ap()

    # k-pair-adjacent spill layouts: [pair][rows][h][N] so MLP gathers are
    # single 3-D DMAs
    FEAT1 = nc.dram_tensor("feat1", [3, 768, 2, N], FP8).ap()   # d1 feats *128
    FEAT2 = nc.dram_tensor("feat2", [3, C * DH, 2, N], FP8).ap()  # d2 feats

    with tile.TileContext(nc) as tc:
        with (
            tc.tile_pool(name="const", bufs=1) as cpool,
            tc.tile_pool(name="nmx", bufs=8) as nmx_pool,
            tc.tile_pool(name="nm1", bufs=10) as nm1_pool,
            tc.tile_pool(name="nm2", bufs=10) as nm2_pool,
            tc.tile_pool(name="wc", bufs=8) as wc_pool,
            tc.tile_pool(name="fm1", bufs=12) as fm1_pool,
            tc.tile_pool(name="fm2", bufs=8) as fm2_pool,
            tc.tile_pool(name="stg", bufs=9) as stg_pool,
            tc.tile_pool(name="feed", bufs=16) as feed_pool,
            tc.tile_pool(name="k0p", bufs=4) as k0_pool,
            tc.tile_pool(name="gw", bufs=2) as gw_pool,
            tc.tile_pool(name="gres", bufs=4) as gres_pool,
            tc.tile_pool(name="ps", bufs=6, space="PSUM") as ps_pool,
            tc.tile_pool(name="pst", bufs=2, space="PSUM") as pst_pool,
        ):
            # ---------------- constants ----------------
            identb = cpool.tile([128, 128], BF16, tag="idb")
            nc.scalar.dma_start(identb[:], IDB[:])
            # resident paired node-major x_h (chain start for both dirs)
            nm_xh = []
            _qs = [nc.sync, nc.scalar, nc.gpsimd]
            for jp in range(NJP):
                t = nmx_pool.tile([128, 2, 768], FP8, name="t", tag="nmx")
                _qs[jp % 3].dma_start(t[:], XHNM[jp])
                nm_xh.append(t)

            def load_wdir(WP):
                """Block-major striped load: all jp's block-0 slices land
                first so the next hop's first 512-col block starts after
                ~1MB instead of the full 4.2MB."""
                ws = []
                qs = [nc.sync, nc.scalar, nc.gpsimd]
                for jp in range(NJP):
                    t = wc_pool.tile([128, 2, N], FP8, name="t", tag="w")
                    ws.append(t)
                for blk in range(NBK):
                    cs = slice(512 * blk, 512 * (blk + 1))
                    for jp in range(NJP):
                        qs[jp % 3].dma_start(ws[jp][:, :, cs],
                                             WP[jp][:, :, cs])
                return ws

            def hop(cur, ws, nch, k, FEATD, kh, rowbase):
                """One DoubleRow hop with fused per-block retransposition.
                Spills fp8 *SF feats to FEATD[:, kh, :]; for k < NHOPS also
                returns the next chain's paired fp8 nm tiles (transposes for
                jp pair (2b, 2b+1) only need block b's columns)."""
                nms = [] if k < NHOPS else None
                scale = 1.0 / 64.0 if k == 1 else 1.0 / 512.0
                for blk in range(NBK):
                    cs = slice(512 * blk, 512 * (blk + 1))
                    pss = [ps_pool.tile([128, 512], F32, name="t", tag="hop")
                           for _ in range(nch)]
                    for jp in range(NJP):
                        rhs = ws[jp][:, :, cs]
                        for c in range(nch):
                            nc.tensor.matmul(
                                pss[c][:],
                                cur[jp][:, :, 128 * c:128 * (c + 1)],
                                rhs,
                                start=(jp == 0), stop=(jp == NJP - 1),
                                perf_mode=DR)
                    fmb = None
                    if nms is not None:
                        pool = fm1_pool if nch == 6 else fm2_pool
                        tg = "fm1" if nch == 6 else "fm2"
                        fmb = [pool.tile([128, 512], BF16, name="t", tag=tg)
                               for _ in range(nch)]
                    for c in range(nch):
                        stg = stg_pool.tile([128, 512], FP8, name="t",
                                            tag="stg")
                        nc.scalar.activation(stg[:], pss[c][:], AF.Copy,
                                             scale=scale)
                        nc.sync.dma_start(
                            FEATD[rowbase + 128 * c:rowbase + 128 * (c + 1),
                                  kh, cs], stg[:])
                        if fmb is not None:
                            nc.vector.tensor_scalar_mul(fmb[c][:], pss[c][:],
                                                        scale)
                    if nms is not None:
                        for q in range(2):
                            t = (nm1_pool.tile([128, 2, 768], FP8, name="t",
                                               tag="nm1") if nch == 6 else
                                 nm2_pool.tile([128, 2, 512], FP8, name="t",
                                               tag="nm2"))
                            for h in range(2):
                                ps = pst_pool.tile([128, 128 * nch], BF16,
                                                   name="t", tag="tr")
                                for c in range(nch):
                                    nc.tensor.transpose(
                                        ps[:, 128 * c:128 * (c + 1)],
                                        fmb[c][:, 128 * (2 * q + h):
                                               128 * (2 * q + h + 1)],
                                        identb[:])
                                nc.vector.tensor_copy(t[:, h, :], ps[:])
                            nms.append(t)
                return nms

            # ---------------- diffusion 1 ----------------
            with nc.named_scope("d1_hops"):
                for dirw, WP in ((0, WFP), (1, WBP)):
                    ws = load_wdir(WP)
                    cur = nm_xh
                    for k in range(1, NHOPS + 1):
                        kidx = dirw * NHOPS + k  # 1..6
                        nxt = hop(cur, ws, 6, k,
                                  FEAT1[(kidx - 1) // 2], (kidx - 1) % 2, 0)
                        if k < NHOPS:
                            cur = nxt

            # ---- deferred small-const loads (weights/biases for MLPs) ----
            brt = cpool.tile([DH, 1], F32, tag="brt")
            nc.scalar.dma_start(brt[:], BRT[:])
            bzt = cpool.tile([DH, 1], F32, tag="bzt")
            nc.scalar.dma_start(bzt[:], BZT[:])
            bnt = cpool.tile([DH, 1], F32, tag="bnt")
            nc.scalar.dma_start(bnt[:], BNT[:])
            wk0a, wk0b, wa, wb = {}, {}, {}, {}
            for g in ("r", "z"):
                wk0a[g] = cpool.tile([128, DH], BF16, name="t", tag=f"w{g}k0a")
                nc.scalar.dma_start(wk0a[g][:], WK0[g][0:128, :])
                wk0b[g] = cpool.tile([64, DH], BF16, name="t", tag=f"w{g}k0b")
                nc.scalar.dma_start(wk0b[g][:], WK0[g][128:192, :])
                wa[g] = []
                wb[g] = []
                for p in range(3):
                    t = cpool.tile([128, 2, DH], FP8, name="t", tag=f"w{g}A{p}")
                    nc.scalar.dma_start(t[:], WA[g][p])
                    wa[g].append(t)
                    t = cpool.tile([64, 2, DH], FP8, name="t", tag=f"w{g}B{p}")
                    nc.scalar.dma_start(t[:], WB[g][p])
                    wb[g].append(t)
            wnk0x = cpool.tile([64, DH], BF16, tag="wnk0x")
            nc.scalar.dma_start(wnk0x[:], WNK0X[:])
            wnk0r = cpool.tile([DH, DH], BF16, tag="wnk0r")
            nc.scalar.dma_start(wnk0r[:], WNK0R[:])
            wnx, wnr = [], []
            for p in range(3):
                t = cpool.tile([64, 2, DH], FP8, name="t", tag=f"wnx{p}")
                nc.scalar.dma_start(t[:], WNX[p])
                wnx.append(t)
                t = cpool.tile([128, 2, DH], FP8, name="t", tag=f"wnr{p}")
                nc.scalar.dma_start(t[:], WNR[p])
                wnr.append(t)


            # ---------------- MLP r,z + rh ----------------
            sc_rz = nc.enter_named_scope("mlp_rz", False)
            z16 = [gres_pool.tile([DH, N], BF16, name="t", tag="z16")
                   for _ in range(C)]
            rh16 = [gres_pool.tile([DH, N], BF16, name="t", tag="rh16")
                    for _ in range(C)]
            for b in range(C):
                for blk in range(NBK):
                    cs = slice(512 * blk, 512 * (blk + 1))
                    k0a = k0_pool.tile([128, 512], BF16, name="t", tag="k0a")
                    nc.gpsimd.dma_start(k0a[:], XHK0[b][0:128, cs])
                    k0b = k0_pool.tile([64, 512], BF16, name="t", tag="k0b")
                    nc.gpsimd.dma_start(k0b[:], XHK0[b][128:192, cs])
                    fA, fB = [], []
                    for p in range(3):
                        tA = feed_pool.tile([128, 2, 512], FP8, name="t",
                                            tag="fA")
                        nc.gpsimd.dma_start(
                            tA[:], FEAT1[p][b * FI:b * FI + 128, :, cs])
                        fA.append(tA)
                        tB = feed_pool.tile([64, 2, 512], FP8, name="t",
                                            tag="fB")
                        nc.scalar.dma_start(
                            tB[:], FEAT1[p][b * FI + 128:b * FI + 192, :, cs])
                        fB.append(tB)
                    hblk = gw_pool.tile([DH, 512], F32, name="t", tag="h")
                    nc.sync.dma_start(hblk[:], HFM[b][:, cs])
                    for g in ("r", "z"):
                        ps = ps_pool.tile([128, 512], F32, name="t", tag="hop")
                        nc.tensor.matmul(ps[:], wk0a[g][:], k0a[:],
                                         start=True, stop=False)
                        nc.tensor.matmul(ps[:], wk0b[g][:], k0b[:],
                                         start=False, stop=False)
                        for p in range(3):
                            nc.tensor.matmul(ps[:], wa[g][p][:], fA[p][:],
                                             start=False, stop=False,
                                             perf_mode=DR)
                            nc.tensor.matmul(ps[:], wb[g][p][:], fB[p][:],
                                             start=False, stop=(p == 2),
                                             perf_mode=DR)
                        if g == "r":
                            rwk = gw_pool.tile([DH, 512], F32, name="t",
                                               tag="rw")
                            nc.scalar.activation(rwk[:], ps[:], AF.Sigmoid,
                                                 bias=brt[:], scale=1.0 / SPS)
                            nc.vector.scalar_tensor_tensor(
                                rh16[b][:, cs], rwk[:], SX, hblk[:],
                                mybir.AluOpType.mult, mybir.AluOpType.mult)
                        else:
                            nc.scalar.activation(z16[b][:, cs], ps[:],
                                                 AF.Sigmoid, bias=bzt[:],
                                                 scale=1.0 / SPS)

            nc.leave_named_scope("mlp_rz", sc_rz[0], False)

            # ---------------- diffusion 2 (rh chain) ----------------
            def build_nm2():
                nms = []
                for jp in range(NJP):
                    ps = pst_pool.tile([128, 2, 512], BF16, name="t", tag="tr")
                    for h in range(2):
                        it = 2 * jp + h
                        for b in range(C):
                            nc.tensor.transpose(
                                ps[:, h, 128 * b:128 * (b + 1)],
                                rh16[b][:, 128 * it:128 * (it + 1)],
                                identb[:])
                    t = nm2_pool.tile([128, 2, 512], FP8, name="t", tag="nm2")
                    nc.vector.tensor_copy(t[:], ps[:])
                    nms.append(t)
                return nms

            with nc.named_scope("d2_hops"):
                for dirw, WP in ((0, WFP), (1, WBP)):
                    ws = load_wdir(WP)
                    cur = build_nm2()
                    for k in range(1, NHOPS + 1):
                        kidx = dirw * NHOPS + k
                        nxt = hop(cur, ws, 4, k,
                                  FEAT2[(kidx - 1) // 2], (kidx - 1) % 2, 0)
                        if k < NHOPS:
                            cur = nxt

            # ---------------- MLP n + final gate ----------------
            sc_n = nc.enter_named_scope("mlp_n", False)
            for b in range(C):
                for blk in range(NBK):
                    cs = slice(512 * blk, 512 * (blk + 1))
                    k0x = k0_pool.tile([64, 512], BF16, name="t", tag="k0b")
                    nc.gpsimd.dma_start(k0x[:], XHK0[b][0:64, cs])
                    fx, fr = [], []
                    for p in range(3):
                        tX = feed_pool.tile([64, 2, 512], FP8, name="t",
                                            tag="fB")
                        nc.scalar.dma_start(
                            tX[:], FEAT1[p][b * FI:b * FI + 64, :, cs])
                        fx.append(tX)
                        tR = feed_pool.tile([128, 2, 512], FP8, name="t",
                                            tag="fA")
                        nc.gpsimd.dma_start(
                            tR[:], FEAT2[p][b * DH:b * DH + 128, :, cs])
                        fr.append(tR)
                    hblk = gw_pool.tile([DH, 512], F32, name="t", tag="h")
                    nc.sync.dma_start(hblk[:], HFM[b][:, cs])
                    ps = ps_pool.tile([128, 512], F32, name="t", tag="hop")
                    nc.tensor.matmul(ps[:], wnk0x[:], k0x[:],
                                     start=True, stop=False)
                    nc.tensor.matmul(ps[:], wnk0r[:], rh16[b][:, cs],
                                     start=False, stop=False)
                    for p in range(3):
                        nc.tensor.matmul(ps[:], wnx[p][:], fx[p][:],
                                         start=False, stop=False, perf_mode=DR)
                        nc.tensor.matmul(ps[:], wnr[p][:], fr[p][:],
                                         start=False, stop=(p == 2),
                                         perf_mode=DR)
                    nf = gw_pool.tile([DH, 512], F32, name="t", tag="nf")
                    nc.scalar.activation(nf[:], ps[:], AF.Tanh, bias=bnt[:],
                                         scale=1.0 / SPS)
                    dlt = gw_pool.tile([DH, 512], F32, name="t", tag="dw")
                    nc.vector.tensor_sub(dlt[:], nf[:], hblk[:])
                    zd = gw_pool.tile([DH, 512], F32, name="t", tag="zd")
                    nc.vector.tensor_mul(zd[:], z16[b][:, cs], dlt[:])
                    o = gw_pool.tile([DH, 512], F32, name="t", tag="o")
                    nc.vector.tensor_add(o[:], zd[:], hblk[:])
                    nc.scalar.dma_start(OUT[b][:, cs], o[:])
            nc.leave_named_scope("mlp_n", sc_n[0], False)

    nc.compile()
    return nc


_NC_CACHE = {}


def _get_nc():
    if "nc" not in _NC_CACHE:
        _NC_CACHE["nc"] = build_nc()
    return _NC_CACHE["nc"]


def _pack_gate_w(W):
    """W [128, 1344] -> (wk0 bf16 [192,128]*8192, wA fp8 [3,128,2,128]*64,
    wB fp8 [3,64,2,128]*64)."""
    W = np.asarray(W, np.float32)
    wk0 = np.ascontiguousarray((W[:, 0:FI].T * SPS)).astype(BF)
    wA = np.zeros((3, 128, 2, DH), np.float32)
    wBt = np.zeros((3, 64, 2, DH), np.float32)
    for p in range(3):
        for h in range(2):
            k = 2 * p + 1 + h
            blkc = W[:, k * FI:(k + 1) * FI]          # [128, 192]
            wA[p, :, h, :] = blkc[:, 0:128].T * SMW
            wBt[p, :, h, :] = blkc[:, 128:192].T * SMW
    return wk0, wA.astype(E4), wBt.astype(E4)


def _pack_n_w(W):
    """Wn [128, 1344] -> k0x bf16 [64,128]*8192, k0r bf16 [128,128]*512,
    wnx fp8 [3,64,2,128]*64, wnr fp8 [3,128,2,128]*64."""
    W = np.asarray(W, np.float32)
    k0x = np.ascontiguousarray(W[:, 0:64].T * SPS).astype(BF)
    k0r = np.ascontiguousarray(W[:, 64:FI].T * (SPS / SX)).astype(BF)
    wnx = np.zeros((3, 64, 2, DH), np.float32)
    wnr = np.zeros((3, 128, 2, DH), np.float32)
    for p in range(3):
        for h in range(2):
            k = 2 * p + 1 + h
            blkc = W[:, k * FI:(k + 1) * FI]
            wnx[p, :, h, :] = blkc[:, 0:64].T * SMW
            wnr[p, :, h, :] = blkc[:, 64:FI].T * SMW
    return k0x, k0r, wnx.astype(E4), wnr.astype(E4)


def _pack_wpair(W):
    """W [N,N] -> fp8 [NJP,128,2,N]: [jp,p,h,i] = 512*W[i, jp*256+h*128+p]."""
    WT = np.asarray(W, np.float32).T * SW                 # [j, i]
    N = WT.shape[0]
    return np.ascontiguousarray(
        WT.reshape(NJP, 2, 128, N).transpose(0, 2, 1, 3)).astype(E4)


def make_in_maps(x, h_prev, W_fwd, W_bwd, Wr, br, Wz, bz, Wn, bn):
    x = np.asarray(x, np.float32)
    h_prev = np.asarray(h_prev, np.float32)
    B, N, Din = x.shape
    wfp = _pack_wpair(W_fwd)
    wbp = _pack_wpair(W_bwd)
    wrk0, wrA, wrB = _pack_gate_w(Wr)
    wzk0, wzA, wzB = _pack_gate_w(Wz)
    wnk0x, wnk0r, wnx, wnr = _pack_n_w(Wn)
    identb = np.eye(128, dtype=np.float32).astype(BF)
    brc = np.ascontiguousarray(np.asarray(br, np.float32).reshape(DH, 1))
    bzc = np.ascontiguousarray(np.asarray(bz, np.float32).reshape(DH, 1))
    bnc = np.ascontiguousarray(np.asarray(bn, np.float32).reshape(DH, 1))
    ncores = B // C
    in_maps = []
    for cix in range(ncores):
        xs = x[C * cix:C * (cix + 1)]
        hs = h_prev[C * cix:C * (cix + 1)]
        xh = np.concatenate([xs, hs], axis=-1)            # [C, N, 192]
        flat = np.ascontiguousarray(xh.transpose(1, 0, 2).reshape(N, C * FI))
        xh_nm8 = np.ascontiguousarray(
            (flat * SX).reshape(NJP, 2, 128, C * FI).transpose(0, 2, 1, 3)
        ).astype(E4)
        xh_k0 = np.ascontiguousarray(xh.transpose(0, 2, 1)).astype(BF)
        h_fm = np.ascontiguousarray(hs.transpose(0, 2, 1))
        in_maps.append(dict(
            xh_nm8=xh_nm8, xh_k0=xh_k0, wfp=wfp, wbp=wbp, h_fm=h_fm,
            wrk0=wrk0, wrA=wrA, wrB=wrB, wzk0=wzk0, wzA=wzA, wzB=wzB,
            wnk0x=wnk0x, wnk0r=wnk0r, wnx=wnx, wnr=wnr,
            br_c=brc, bz_c=bzc, bn_c=bnc, identb=identb))
    return in_maps, ncores


def kernel(x, h_prev, W_fwd, W_bwd, Wr, br, Wz, bz, Wn, bn, _trace=False):
    in_maps, ncores = make_in_maps(
        x, h_prev, W_fwd, W_bwd, Wr, br, Wz, bz, Wn, bn)
    nc = _get_nc()
    res = run_bass_kernel_spmd(nc, in_maps, list(range(ncores)), trace=_trace)
    outs = [np.ascontiguousarray(res.results[c]["out_fm"].transpose(0, 2, 1))
            for c in range(ncores)]
    full = np.concatenate(outs, axis=0).astype(np.float32)
    if _trace:
        return full, res
    return full
